# revision 83
# baseline (speedup 1.0000x reference)
"""Trainium2 Bass kernel for nn_BertLayer_47339129536519.

BertLayer with hierarchical dialog attention:
  1) token-level MHA + SelfOutput(LN)       [B=32, S=512, H=768, 12 heads]
  2) dialog attention over per-turn CLS tokens (4 dialogs x 8 turns)
  3) FFN (gelu-erf) + output LN

Sharding: data-parallel over the 32 sequences, 4 per core on 8 cores.
The dialog attention mixes CLS vectors across cores -> tiny AllGather
(32x768) and every core redundantly computes the (tiny) dialog block.

v9 vs v2 (the 885us baseline):
  * the chip runs power-throttled (avg tensor-util limit ~70%), so
    wall time ~ PE-busy / 0.7: every tensor-engine cycle cut pays 1.4x.
  * fp8e4 DoubleRow matmuls (2 k-tiles per pass ~ 2x bf16 FLOPs) for
    the V/Q/K/AO projections (plain fp8: softmax+LN+residual damp the
    quantization to ~1e-3 of the output), for Wo2 (fp8 interT), and
    for 4 of Wi's 6 k-chunks (hybrid: error scales sqrt(2/3)).
    Weights pre-scaled x32 into e4m3's normal range; the scale rides
    psum and is folded into gelu/exp input scales and LN scale
    invariance (x carries x1024 via the host, x1 carries x32).
  * phase 1 software-pipelined: projections of seq s+1 interleave with
    attention (scores/exp/PV/AO/LN) of seq s via weighted round-robin
    generator emission, keeping the PE fed while ACT chews exp.
  * LayerNorm rstd: ACT Sqrt + reciprocal_approx_fast + one Newton
    step (the raw approx's ~4e-3 rstd error scales the output 1:1).
  * dialog attention emitted interleaved with FFN(seq2) chunks so its
    skinny dependency chain doesn't head-of-line-block the PE queue;
    CLS fixup rides FFN(seq3) (stale-CLS trick for columns 1..S-1).
  * qt bias + softmax denominator copies on DVE (ACT is exp-bound).
"""

import numpy as np

import concourse.bass as bass
import concourse.mybir as mybir
import concourse.tile as tile
from concourse import bacc
from concourse.bass_utils import run_bass_kernel_spmd

HID, NH, HD, S = 768, 12, 64, 512
B, NCORES, SPC = 32, 8, 4  # batch, cores, sequences per core
TURNS = 8
NDLG = B // TURNS  # 4 dialogs
HC = HID // 128  # 6 hidden-dim chunks of 128
IC = (4 * HID) // 128  # 24 intermediate chunks
INTER = 4 * HID  # 3072
EPS = 1e-12
ISCALE = 0.125  # 1/sqrt(64)

F32 = mybir.dt.float32
F32R = mybir.dt.float32r
BF = mybir.dt.bfloat16
F8 = mybir.dt.float8e4
DR = mybir.MatmulPerfMode.DoubleRow
AF = mybir.ActivationFunctionType
ALU = mybir.AluOpType
AX = mybir.AxisListType
WSC = 32.0  # fp8 weight pre-scale (0.02-sigma weights -> normal e4m3 range)


def _drive(*gens, weights=None):
    """Weighted round-robin drive: interleaves generator emission so
    independent work lands between dependent chains in each engine's
    (in-order) queue.  weights[i] = how many steps of gens[i] per cycle
    (fractional allowed: 0.5 = one step every other cycle)."""
    live = [(g, (weights[i] if weights else 1.0))
            for i, g in enumerate(gens) if g is not None]
    credit = [0.0] * len(live)
    while live:
        for i, (g, w) in enumerate(list(live)):
            if g is None:
                continue
            credit[i] += w
            while credit[i] >= 1.0 and g is not None:
                credit[i] -= 1.0
                try:
                    next(g)
                except StopIteration:
                    live[i] = (None, w)
                    g = None
        if all(g is None for g, _ in live):
            break


def _ln_reps(nc, rows, lnp_m, lnp_q, eps_t, n, dim, oscale=1.0):
    """From accumulated sum (lnp_m[1,n]) / sum-of-squares (lnp_q[1,n]) psum
    rows, produce broadcast [128, n] tiles (rstd_rep, mscaled_rep) so that
    oscale*normalized = y * rstd_rep - mscaled_rep.  The reciprocal is
    approx_fast + one Newton step (error ~(4e-3)^2, vs 4e-3 for the raw
    approx, which directly scales the LN output).  oscale folds into the
    Newton bracket for free.  LN is scale-invariant in y, so callers can
    feed pre-scaled y without adjusting anything here."""
    # scratch rows packed on partitions of one tile: [1,n] tiles cost a
    # full 2KB of per-partition address space each.  mean/rstd stay
    # partition-0 tiles (partition_broadcast reads partition 0).
    # (two SBUF inputs of a DVE op must share base partition -> keep all
    #  row tiles at partition 0; fold intermediates in place)
    mean = rows.tile([1, n], F32, tag="ln_mean", bufs=1)
    nc.vector.tensor_scalar_mul(mean[:], lnp_m[:], 1.0 / dim)
    rstd = rows.tile([1, n], F32, tag="ln_rstd", bufs=1)
    # rstd holds mean^2 transiently
    nc.vector.tensor_tensor(out=rstd[:], in0=mean[:], in1=mean[:], op=ALU.mult)
    var = rows.tile([1, n], F32, tag="ln_var", bufs=1)
    nc.vector.scalar_tensor_tensor(
        out=var[:], in0=lnp_q[:], scalar=1.0 / dim, in1=rstd[:],
        op0=ALU.mult, op1=ALU.subtract,
    )
    nc.scalar.activation(var[:], var[:], AF.Sqrt, bias=eps_t[:])
    r0 = rows.tile([1, n], F32, tag="ln_r0", bufs=1)
    nc.vector.reciprocal_approx_fast(out=r0[:], in_=var[:])
    # Newton: rstd = r0 * (2 - var * r0); var becomes the bracket in place
    nc.vector.tensor_tensor(out=var[:], in0=var[:], in1=r0[:], op=ALU.mult)
    nc.vector.tensor_scalar(
        out=var[:], in0=var[:], scalar1=-oscale, scalar2=2.0 * oscale,
        op0=ALU.mult, op1=ALU.add,
    )
    nc.vector.tensor_tensor(out=rstd[:], in0=r0[:], in1=var[:], op=ALU.mult)
    nc.vector.tensor_tensor(out=mean[:], in0=mean[:], in1=rstd[:], op=ALU.mult)
    rstd_rep = rows.tile([128, n], F32, tag="ln_rstd_rep", bufs=1)
    nc.gpsimd.partition_broadcast(rstd_rep[:], rstd[:])
    msc_rep = rows.tile([128, n], F32, tag="ln_msc_rep", bufs=1)
    nc.gpsimd.partition_broadcast(msc_rep[:], mean[:])
    return rstd_rep, msc_rep


def _emit(tc, d):
    nc = tc.nc
    from concourse import library_config

    nc.gpsimd.load_library(library_config.attn)  # for partition_broadcast

    with (
        tc.tile_pool(name="setup", bufs=1) as setup,
        tc.tile_pool(name="rows", bufs=2) as rows,
        tc.tile_pool(name="dram", bufs=1, space="DRAM") as dram,
    ):
        # ---- constants / small params ----
        ones_f32 = setup.tile([128, 2], F32)
        nc.vector.memset(ones_f32, 1.0)
        ones_fr = ones_f32.bitcast(F32R)
        ones_bf = setup.tile([128, 2], BF)
        nc.vector.memset(ones_bf, 1.0)
        eps_t = setup.tile([1, 1], F32)
        nc.vector.memset(eps_t, EPS)

        def load_small(name, dt=F32):
            t = setup.tile(list(d[name].shape), dt, name="sb_" + name)
            nc.sync.dma_start(t[:], d[name][:])
            return t

        bq_s = load_small("bq")
        bk_s = load_small("bk")
        bao_s = load_small("bao")
        bv_rep = load_small("bv_rep", BF)
        dbq_s = load_small("dbq")
        dbk_s = load_small("dbk")
        dbo_s = load_small("dbo")
        bi_s = load_small("bi")
        bo2_s = load_small("bo2")

        # persistent-through-kernel tiles.  x1 holds 32*LN1out (bf16 is
        # scale-free; LN2 washes the factor out).  x1q: fp8 copy of hid
        # chunks 0-3 for the hybrid-precision Wi GEMM (4 chunks fp8
        # DoubleRow + 2 chunks bf16 -> 2/3 of the x/W quantization noise
        # at 2/3 of the bf16 cycles).
        x1 = setup.tile([128, SPC, HC, S], BF)
        x1q = setup.tile([128, SPC, 4, S], F8)
        cls_in = dram.tile([128, HC, SPC], F32, name="cls_in")
        cls_out = dram.tile([NCORES * 128, HC, SPC], F32, name="cls_out")

        # FFN Wi weights: resident, DMA overlapped with phase 1
        with tc.tile_pool(name="ffw", bufs=1) as ffw:
          wi8_s = ffw.tile([128, IC, 4, 128], F8)
          wib_s = ffw.tile([128, IC, 2, 128], BF)
          # ========================= PHASE 1: token attention =================
          with (
            tc.tile_pool(name="attw", bufs=1) as attw,
            tc.tile_pool(name="attp", bufs=1) as attp,
            tc.tile_pool(name="psP", bufs=2, space="PSUM") as psP,
            tc.tile_pool(name="psS", bufs=2, space="PSUM") as psS,
            tc.tile_pool(name="psC", bufs=2, space="PSUM") as psC,
            tc.tile_pool(name="psL", bufs=1, space="PSUM") as psL,
          ):
            # attention weights in plain fp8 (WSC-scaled): softmax + LN +
            # residual damp the quantization to ~1e-3 of the output, and
            # the DoubleRow matmuls run the projections at 2x bf16 rate.
            attws = {}
            for nm in ["wv", "wq", "wk", "wao"]:
                t = attw.tile([128, HC, HID], F8, name="sb_" + nm)
                nc.sync.dma_start(t[:], d[nm + "_hi"][:])
                attws[nm] = t
            for g in range(8):
                nc.sync.dma_start(
                    wi8_s[:, g * 3 : (g + 1) * 3, :, :],
                    d["wi8"][:, g * 3 : (g + 1) * 3],
                )
                nc.sync.dma_start(
                    wib_s[:, g * 3 : (g + 1) * 3, :, :],
                    d["wib"][:, g * 3 : (g + 1) * 3],
                )

            xtiles = {}

            def load_x(si):
                if si >= SPC:
                    return
                t = attp.tile([128, HC, S], BF, tag="xs", bufs=2, name="xs")
                for g in range(3):
                    nc.sync.dma_start(
                        t[:, 2 * g : 2 * g + 2, :], d["xbf"][si, :, 2 * g : 2 * g + 2]
                    )
                t8 = attp.tile([128, HC, S], F8, tag="xs8", bufs=2, name="xs8")
                nc.sync.dma_start(t8[:], d["x8"][si])
                xtiles[si] = (t, t8)

            # per-seq projection outputs (2 generations live: s and s+1)
            vtiles, qtiles, ktiles = {}, {}, {}

            def proj_stage(s):
                """V/Q/K projections for seq s: fp8 DoubleRow over hc pairs,
                hi then lo weight halves accumulating into one psum chain.
                psum comes out at WSC*(x@W); the bias evictions rescale."""
                xs, xs8 = xtiles[s]
                v_aug = attp.tile([128, 4, NH, HD + 1], BF, tag="vaug", bufs=2)
                nc.vector.memset(v_aug[:, :, :, HD : HD + 1], 1.0)
                vtiles[s] = v_aug
                GH = HC // 2
                wv8 = attws["wv"]
                for sc in range(4):
                    for half in range(2):
                        pv = psP.tile([128, S], F32, tag="psP")
                        for g in range(GH):
                            nc.tensor.matmul(
                                pv[:, :384],
                                xs8[:, 2 * g : 2 * g + 2,
                                    sc * 128 : (sc + 1) * 128],
                                wv8[:, 2 * g : 2 * g + 2,
                                    half * 384 : (half + 1) * 384],
                                start=(g == 0),
                                stop=(g == GH - 1),
                                perf_mode=DR,
                            )
                        nc.vector.tensor_tensor(
                            out=v_aug[:, sc, half * 6 : half * 6 + 6, 0:HD],
                            in0=pv[:, :384].rearrange("p (h e) -> p h e", e=HD),
                            in1=bv_rep[
                                :, half * 384 : (half + 1) * 384
                            ].rearrange("p (h e) -> p h e", e=HD),
                            op=ALU.add,
                        )
                        yield
                qt = attp.tile([128, HC, S], F8, tag="qt", bufs=2)
                kt = attp.tile([128, HC, S], F8, tag="kt", bufs=2)
                qtiles[s], ktiles[s] = qt, kt
                wq8, wk8 = attws["wq"], attws["wk"]
                for dc in range(HC):
                    pq = psP.tile([128, S], F32, tag="psP")
                    for g in range(GH):
                        nc.tensor.matmul(
                            pq[:],
                            wq8[:, 2 * g : 2 * g + 2, dc * 128 : (dc + 1) * 128],
                            xs8[:, 2 * g : 2 * g + 2, :],
                            start=(g == 0),
                            stop=(g == GH - 1),
                            perf_mode=DR,
                        )
                    nc.vector.tensor_scalar_add(
                        out=qt[:, dc, :], in0=pq[:], scalar1=bq_s[:, dc : dc + 1]
                    )
                    yield
                    pk = psP.tile([128, S], F32, tag="psP")
                    for g in range(GH):
                        nc.tensor.matmul(
                            pk[:],
                            wk8[:, 2 * g : 2 * g + 2, dc * 128 : (dc + 1) * 128],
                            xs8[:, 2 * g : 2 * g + 2, :],
                            start=(g == 0),
                            stop=(g == GH - 1),
                            perf_mode=DR,
                        )
                    nc.vector.tensor_scalar_add(
                        out=kt[:, dc, :], in0=pk[:], scalar1=bk_s[:, dc : dc + 1]
                    )
                    yield

            def attn_stage(s):
                """scores/softmax/PV/AO/LN1 for seq s (needs proj_stage(s)
                complete). ACT(exp)-dominated; meant to overlap
                proj_stage(s+1) on the PE."""
                xs, _xs8 = xtiles.pop(s)
                v_aug = vtiles.pop(s)
                qt = qtiles.pop(s)
                kt = ktiles.pop(s)
                load_x(s + 2)

                # ctxT carries WSC*ctx (v_aug holds WSC*v with a 1.0 ones
                # column, so the PV ratio comes out WSC-scaled), fp8 for
                # the DoubleRow AO projection.
                ctxT = attp.tile([128, HC, S], F8, tag="ctxT", bufs=1)

                def emit_scores(dc):
                    probsT = attp.tile(
                        [128, 4, 2, S], BF, tag="probsT", bufs=2, name="probsT"
                    )
                    for kc in range(4):
                        pse = psS.tile([128, S], F32, tag="ps_s")
                        pso = psS.tile([128, S], F32, tag="ps_s")
                        nc.tensor.matmul(
                            pse[:],
                            kt[0:64, dc, kc * 128 : (kc + 1) * 128],
                            qt[0:64, dc, :],
                            start=True, stop=True,
                        )
                        nc.tensor.matmul(
                            pso[:],
                            kt[64:128, dc, kc * 128 : (kc + 1) * 128],
                            qt[64:128, dc, :],
                            start=True, stop=True,
                        )
                        nc.scalar.activation(
                            probsT[:, kc, 0, :], pse[:], AF.Exp,
                            scale=ISCALE / (WSC * WSC),
                        )
                        nc.scalar.activation(
                            probsT[:, kc, 1, :], pso[:], AF.Exp,
                            scale=ISCALE / (WSC * WSC),
                        )
                    return probsT

                def emit_pv(dc, probsT, eo):
                    h = 2 * dc + eo
                    pc = psC.tile([HD + 1, S], F32, tag="pc")
                    for kc in range(4):
                        nc.tensor.matmul(
                            pc[:],
                            v_aug[:, kc, h, :],
                            probsT[:, kc, eo, :],
                            start=(kc == 0),
                            stop=(kc == 3),
                        )
                    dn = rows.tile([1, S], F32, tag="dn", bufs=2)
                    nc.vector.tensor_copy(out=dn[:], in_=pc[HD : HD + 1, :])
                    rcp = rows.tile([1, S], F32, tag="rcp", bufs=2)
                    nc.vector.reciprocal_approx_fast(out=rcp[:], in_=dn[:])
                    rep = rows.tile([HD, S], F32, tag="rep")
                    nc.gpsimd.partition_broadcast(rep[:], rcp[:])
                    nc.vector.tensor_tensor(
                        out=ctxT[eo * 64 : eo * 64 + 64, dc, :],
                        in0=pc[0:HD, :],
                        in1=rep[:],
                        op=ALU.mult,
                    )

                prev = None
                for dc in range(HC):
                    pt = emit_scores(dc)
                    yield
                    if prev is not None:
                        emit_pv(prev[0], prev[1], 0)
                        yield
                        emit_pv(prev[0], prev[1], 1)
                        yield
                    prev = (dc, pt)
                emit_pv(prev[0], prev[1], 0)
                yield
                emit_pv(prev[0], prev[1], 1)
                yield

                # ---- attention output projection + residual + LN1 ----
                y = attp.tile([128, HC, S], F32R, tag="y")
                lnp_m = psL.tile([1, S], F32, tag="lnm")
                lnp_q = psL.tile([1, S], F32, tag="lnq")
                GH = HC // 2
                wao8 = attws["wao"]
                for dc in range(HC):
                    pa = psP.tile([128, S], F32, tag="psP")
                    for g in range(GH):
                        nc.tensor.matmul(
                            pa[:],
                            wao8[:, 2 * g : 2 * g + 2, dc * 128 : (dc + 1) * 128],
                            ctxT[:, 2 * g : 2 * g + 2, :],
                            start=(g == 0),
                            stop=(g == GH - 1),
                            perf_mode=DR,
                        )
                    nc.vector.scalar_tensor_tensor(
                        out=y[:, dc, :], in0=pa[:], scalar=bao_s[:, dc : dc + 1],
                        in1=xs[:, dc, :], op0=ALU.add, op1=ALU.add,
                    )
                    sq = attp.tile([128, S], BF, tag="sq", bufs=1)
                    nc.vector.tensor_tensor(
                        out=sq[:], in0=y[:, dc, :], in1=y[:, dc, :], op=ALU.mult
                    )
                    nc.tensor.matmul(
                        lnp_m[:], ones_fr[:, 0:1], y[:, dc, :],
                        start=(dc == 0), stop=(dc == HC - 1),
                        skip_group_check=True,
                    )
                    nc.tensor.matmul(
                        lnp_q[:], ones_bf[:, 0:1], sq[:],
                        start=(dc == 0), stop=(dc == HC - 1),
                        skip_group_check=True,
                    )
                    yield
                rstd_rep, msc_rep = _ln_reps(
                    nc, rows, lnp_m, lnp_q, eps_t, S, HID, oscale=WSC
                )
                for dc in range(HC):
                    nc.vector.tensor_tensor(
                        out=x1[:, s, dc, :], in0=y[:, dc, :], in1=rstd_rep[:],
                        op=ALU.mult,
                    )
                    nc.vector.tensor_tensor(
                        out=x1[:, s, dc, :], in0=x1[:, s, dc, :], in1=msc_rep[:],
                        op=ALU.subtract,
                    )
                    if dc < 4:
                        nc.vector.tensor_copy(
                            out=x1q[:, s, dc, :], in_=x1[:, s, dc, :]
                        )
                    yield
                clsst = rows.tile([128, HC, 1], F32, tag="clsst")
                nc.vector.tensor_copy(out=clsst[:], in_=x1[:, s, :, 0:1])
                nc.sync.dma_start(cls_in[:, :, s : s + 1], clsst[:])

            load_x(0)
            load_x(1)
            _drive(proj_stage(0))
            for s in range(SPC):
                # ~30 attn steps vs 20 proj steps: drive proj at 2/3 rate
                # so its (PE-dense) chunks stretch across the whole
                # (ACT-bound) attn span instead of exhausting early.
                _drive(
                    attn_stage(s),
                    proj_stage(s + 1) if s + 1 < SPC else None,
                    weights=[1.0, 0.67],
                )

          # ========= PHASE 2+3: dialog attention overlapped with FFN =========
            #
            # FFN for all 4 seqs runs on x1 with the *stale* CLS column; every
            # token's FFN+LN2 is independent, so only column 0 of each output
            # is affected - and column 0 is not stored from the main pass.
            # The dialog block (gather + tiny attention) is emitted interleaved
            # with seq 2's FFN chunks, and a CLS fixup pass (all 32 updated
            # CLS vectors, N=32 moving) rides along seq 3's FFN loops.
          with (
                tc.tile_pool(name="dlgw", bufs=1) as dlgw,
                tc.tile_pool(name="dlgp", bufs=1) as dlgp,
                tc.tile_pool(name="ffp", bufs=1) as ffp,
                tc.tile_pool(name="psZ", bufs=2, space="PSUM") as psZ,
                tc.tile_pool(name="psO", bufs=2, space="PSUM") as psO,
                tc.tile_pool(name="psL2", bufs=1, space="PSUM") as psL2,
                tc.tile_pool(name="psD", bufs=1, space="PSUM") as psD,
          ):
                nc.gpsimd.collective_compute(
                    "AllGather",
                    ALU.bypass,
                    replica_groups=[list(range(NCORES))],
                    ins=[cls_in.opt()],
                    outs=[cls_out.opt()],
                )
                dwq_s = dlgw.tile([128, HC, HID], BF, tag="dw", bufs=2)
                nc.sync.dma_start(dwq_s[:], d["dwq"][:])
                dwk_s = dlgw.tile([128, HC, HID], BF, tag="dw", bufs=2)
                nc.sync.dma_start(dwk_s[:], d["dwk"][:])
                dbv_rep = dlgw.tile([128, HID], BF, name="sb_dbv_rep")
                nc.sync.dma_start(dbv_rep[:], d["dbv_rep"][:])
                cmaskt_s = dlgw.tile([B, NH, B], F32, name="sb_cmaskt")
                nc.sync.dma_start(cmaskt_s[:], d["cmaskt"][:])

                def ffn_seq(s, fix):
                    x2 = fix  # x2clsT (bf16, WSC-scaled) when fix is set
                    # Wi runs in bf16 (same cycles as a split-fp8 pair, but
                    # no x/W quantization error); psum = WSC*(x@Wi) since x1
                    # carries WSC -> fold 1/WSC into the gelu input scale.
                    # Wo2 runs fp8 DoubleRow with hi+lo split weights; only
                    # the fp8 interT quantization error remains.
                    interT = ffp.tile([128, IC, S], F8, tag="interT")
                    if fix:
                        interC = ffp.tile([128, IC, B], F8, tag="interC")
                    for ic in range(IC):
                        pz = psZ.tile([128, S], F32, tag="pz")
                        if fix:
                            pzc = psZ.tile([128, B], F32, tag="pz")
                        for g in range(2):
                            nc.tensor.matmul(
                                pz[:], wi8_s[:, ic, 2 * g : 2 * g + 2, :],
                                x1q[:, s, 2 * g : 2 * g + 2, :],
                                start=(g == 0), stop=False, perf_mode=DR,
                            )
                            if fix:
                                nc.tensor.matmul(
                                    pzc[:], wi8_s[:, ic, 2 * g : 2 * g + 2, :],
                                    x2q[:, 2 * g : 2 * g + 2, :],
                                    start=(g == 0), stop=False, perf_mode=DR,
                                )
                        for hb in range(2):
                            nc.tensor.matmul(
                                pz[:], wib_s[:, ic, hb, :], x1[:, s, 4 + hb, :],
                                start=False, stop=(hb == 1),
                            )
                            if fix:
                                nc.tensor.matmul(
                                    pzc[:], wib_s[:, ic, hb, :], x2[:, 4 + hb, :],
                                    start=False, stop=(hb == 1),
                                )
                        nc.scalar.activation(
                            interT[:, ic, :], pz[:], AF.Gelu,
                            bias=bi_s[:, ic : ic + 1], scale=1.0 / (WSC * WSC),
                        )
                        if fix:
                            nc.scalar.activation(
                                interC[:, ic, :], pzc[:], AF.Gelu,
                                bias=bi_s[:, ic : ic + 1],
                                scale=1.0 / (WSC * WSC),
                            )
                        if ic % 2 == 1:
                            yield
                    y2 = ffp.tile([128, HC, S], F32R, tag="y2")
                    lnp2_m = psL2.tile([1, S], F32, tag="lnm")
                    lnp2_q = psL2.tile([1, S], F32, tag="lnq")
                    if fix:
                        y2c = ffp.tile([128, HC, B], F32R, tag="y2c")
                        lnc_m = psD.tile([1, B], F32, tag="pd")
                        lnc_q = psD.tile([1, B], F32, tag="pdo")
                    GI = IC // 2
                    for oc in range(HC):
                        wo2_sl = ffp.tile([128, IC, 128], F8, tag="wo2", bufs=2)
                        for g in range(3):
                            nc.sync.dma_start(
                                wo2_sl[:, 8 * g : 8 * g + 8, :],
                                d["wo2_hi"][:, oc, 8 * g : 8 * g + 8],
                            )
                        po = psO.tile([128, S], F32, tag="po")
                        if fix:
                            poc = psO.tile([128, B], F32, tag="po")
                        for g in range(GI):
                            nc.tensor.matmul(
                                po[:],
                                wo2_sl[:, 2 * g : 2 * g + 2, :],
                                interT[:, 2 * g : 2 * g + 2, :],
                                start=(g == 0),
                                stop=(g == GI - 1),
                                perf_mode=DR,
                            )
                            if fix:
                                nc.tensor.matmul(
                                    poc[:],
                                    wo2_sl[:, 2 * g : 2 * g + 2, :],
                                    interC[:, 2 * g : 2 * g + 2, :],
                                    start=(g == 0),
                                    stop=(g == GI - 1),
                                    perf_mode=DR,
                                )
                        nc.vector.scalar_tensor_tensor(
                            out=y2[:, oc, :], in0=po[:],
                            scalar=bo2_s[:, oc : oc + 1], in1=x1[:, s, oc, :],
                            op0=ALU.add, op1=ALU.add,
                        )
                        fsq = ffp.tile([128, S], F32R, tag="fsq", bufs=2)
                        nc.vector.tensor_tensor(
                            out=fsq[:], in0=y2[:, oc, :], in1=y2[:, oc, :],
                            op=ALU.mult,
                        )
                        nc.tensor.matmul(
                            lnp2_m[:], ones_fr[:, 0:1], y2[:, oc, :],
                            start=(oc == 0), stop=(oc == HC - 1),
                            skip_group_check=True,
                        )
                        nc.tensor.matmul(
                            lnp2_q[:], ones_fr[:, 0:1], fsq[:],
                            start=(oc == 0), stop=(oc == HC - 1),
                            skip_group_check=True,
                        )
                        if fix:
                            nc.vector.scalar_tensor_tensor(
                                out=y2c[:, oc, :], in0=poc[:],
                                scalar=bo2_s[:, oc : oc + 1], in1=x2[:, oc, :],
                                op0=ALU.add, op1=ALU.add,
                            )
                            fsqc = ffp.tile([128, B], F32R, tag="fsqc", bufs=2)
                            nc.vector.tensor_tensor(
                                out=fsqc[:], in0=y2c[:, oc, :], in1=y2c[:, oc, :],
                                op=ALU.mult,
                            )
                            nc.tensor.matmul(
                                lnc_m[:], ones_fr[:, 0:1], y2c[:, oc, :],
                                start=(oc == 0), stop=(oc == HC - 1),
                                skip_group_check=True,
                            )
                            nc.tensor.matmul(
                                lnc_q[:], ones_fr[:, 0:1], fsqc[:],
                                start=(oc == 0), stop=(oc == HC - 1),
                                skip_group_check=True,
                            )
                        yield
                    rstd_rep, msc_rep = _ln_reps(
                        nc, rows, lnp2_m, lnp2_q, eps_t, S, HID
                    )
                    outst = ffp.tile([128, HC, S], F32, tag="outst", bufs=2)
                    for oc in range(HC):
                        nc.vector.tensor_tensor(
                            out=outst[:, oc, :], in0=y2[:, oc, :],
                            in1=rstd_rep[:], op=ALU.mult,
                        )
                        nc.vector.tensor_tensor(
                            out=outst[:, oc, :], in0=outst[:, oc, :],
                            in1=msc_rep[:], op=ALU.subtract,
                        )
                        if oc % 2 == 1:
                            yield
                    for g in range(3):
                        nc.sync.dma_start(
                            d["out"][s][:, 2 * g : 2 * g + 2, 1:S],
                            outst[:, 2 * g : 2 * g + 2, 1:S],
                        )
                    if fix:
                        rsc, msc = _ln_reps(nc, rows, lnc_m, lnc_q, eps_t, B, HID)
                        outc = ffp.tile([128, HC, B], F32, tag="outc")
                        for oc in range(HC):
                            nc.vector.tensor_tensor(
                                out=outc[:, oc, :], in0=y2c[:, oc, :],
                                in1=rsc[:], op=ALU.mult,
                            )
                            nc.vector.tensor_tensor(
                                out=outc[:, oc, :], in0=outc[:, oc, :],
                                in1=msc[:], op=ALU.subtract,
                            )
                        outcl = ffp.tile([128, HC, 1, SPC], F32, tag="outcl")
                        pid = nc.partition_id()
                        nc.vector.tensor_copy(
                            out=outcl[:],
                            in_=outc.rearrange("p c (r s) -> p c r s", s=SPC)[
                                :, :, bass.ds(pid, 1), :
                            ],
                        )
                        for s2 in range(SPC):
                            nc.sync.dma_start(
                                d["out"][s2][:, :, 0:1], outcl[:, :, 0, s2 : s2 + 1]
                            )

                def dialog_stage():
                    # -------------------- dialog attention (tiny) ------------
                    clsF = dlgp.tile([128, HC, B], F32)
                    for r in range(NCORES):
                        nc.sync.dma_start(
                            clsF[:, :, r * SPC : (r + 1) * SPC],
                            cls_out[r * 128 : (r + 1) * 128, :, :],
                        )
                    clsT = dlgp.tile([128, HC, B], BF)
                    nc.vector.tensor_copy(out=clsT[:], in_=clsF[:])
                    yield

                    qdT = dlgp.tile([128, HC, B], BF)
                    kdT = dlgp.tile([128, HC, B], BF)
                    for dc in range(HC):
                        pq = psD.tile([128, B], F32, tag="pd")
                        for hc in range(HC):
                            nc.tensor.matmul(
                                pq[:], dwq_s[:, hc, dc * 128 : (dc + 1) * 128],
                                clsT[:, hc, :], start=(hc == 0), stop=(hc == HC - 1),
                            )
                        nc.vector.tensor_scalar_add(
                            out=qdT[:, dc, :], in0=pq[:], scalar1=dbq_s[:, dc : dc + 1]
                        )
                        yield
                        pk = psD.tile([128, B], F32, tag="pdo")
                        for hc in range(HC):
                            nc.tensor.matmul(
                                pk[:], dwk_s[:, hc, dc * 128 : (dc + 1) * 128],
                                clsT[:, hc, :], start=(hc == 0), stop=(hc == HC - 1),
                            )
                        nc.vector.tensor_scalar_add(
                            out=kdT[:, dc, :], in0=pk[:], scalar1=dbk_s[:, dc : dc + 1]
                        )
                        yield
                    dwv_s = dlgw.tile([128, HC, HID], BF, tag="dw", bufs=2)
                    nc.sync.dma_start(dwv_s[:], d["dwv"][:])
                    dwo_s = dlgw.tile([128, HC, HID], BF, tag="dw", bufs=2)
                    nc.sync.dma_start(dwo_s[:], d["dwo"][:])
                    # v natural [32, 768] + ones column per head
                    vd_aug = dlgp.tile([B, NH, HD + 1], BF)
                    nc.vector.memset(vd_aug[:, :, HD : HD + 1], 1.0)
                    for half in range(2):
                        pv = psD.tile([B, 384], F32, tag="pd")
                        for hc in range(HC):
                            nc.tensor.matmul(
                                pv[:], clsT[:, hc, :],
                                dwv_s[:, hc, half * 384 : (half + 1) * 384],
                                start=(hc == 0), stop=(hc == HC - 1),
                            )
                        nc.vector.tensor_tensor(
                            out=vd_aug[:, half * 6 : half * 6 + 6, 0:HD],
                            in0=pv[:].rearrange("p (h e) -> p h e", e=HD),
                            in1=dbv_rep[:B, half * 384 : (half + 1) * 384].rearrange(
                                "p (h e) -> p h e", e=HD
                            ),
                            op=ALU.add,
                        )
                        yield

                    # transposed scores: per-head matmuls (ping-pong between the
                    # two dialog psum banks), mask+scale on DVE, one batched exp.
                    sdt = dlgp.tile([B, NH, B], F32)
                    for h in range(NH):
                        dc, off = h // 2, (h % 2) * 64
                        pss = psD.tile([B, B], F32, tag=("pdo" if h % 2 == 0 else "pd"))
                        nc.tensor.matmul(
                            pss[:], kdT[off : off + 64, dc, :],
                            qdT[off : off + 64, dc, :], start=True, stop=True,
                        )
                        nc.vector.scalar_tensor_tensor(
                            out=sdt[:, h, :], in0=pss[:],
                            scalar=ISCALE / (WSC * WSC),
                            in1=cmaskt_s[:, h, :], op0=ALU.mult, op1=ALU.add,
                        )
                        if h % 3 == 2:
                            yield
                    probsTd = dlgp.tile([B, NH, B], BF)
                    nc.scalar.activation(probsTd[:], sdt[:], AF.Exp)
                    yield
                    ctxdT = dlgp.tile([128, HC, B], BF)
                    for h in range(NH):
                        dc, off = h // 2, (h % 2) * 64
                        pcd = psD.tile(
                            [HD + 1, B], F32, tag=("pdo" if h % 2 == 0 else "pd")
                        )
                        nc.tensor.matmul(
                            pcd[:], vd_aug[:, h, :], probsTd[:, h, :],
                            start=True, stop=True,
                        )
                        dnd = rows.tile([1, B], F32, tag="dnd", bufs=2)
                        nc.vector.tensor_copy(out=dnd[:], in_=pcd[HD : HD + 1, :])
                        rcpd = rows.tile([1, B], F32, tag="rcpd", bufs=2)
                        nc.vector.reciprocal_approx_fast(out=rcpd[:], in_=dnd[:])
                        repd = rows.tile([HD, B], F32, tag="repd", bufs=2)
                        nc.gpsimd.partition_broadcast(repd[:], rcpd[:])
                        nc.vector.tensor_tensor(
                            out=ctxdT[off : off + 64, dc, :],
                            in0=pcd[0:HD, :], in1=repd[:], op=ALU.mult,
                        )
                        if h % 3 == 2:
                            yield

                    # dialog output projection + residual + LN
                    # (po shares the psO banks with the FFN po rotation;
                    #  the LN stat accumulators take the psD ping-pong banks,
                    #  which have no other users from here to the fixup.)
                    ydT = dlgp.tile([128, HC, B], F32R)
                    lnpd_m = psD.tile([1, B], F32, tag="pd")
                    lnpd_q = psD.tile([1, B], F32, tag="pdo")
                    for oc in range(HC):
                        po = psO.tile([128, B], F32, tag="po")
                        for hc in range(HC):
                            nc.tensor.matmul(
                                po[:], dwo_s[:, hc, oc * 128 : (oc + 1) * 128],
                                ctxdT[:, hc, :], start=(hc == 0), stop=(hc == HC - 1),
                            )
                        nc.vector.scalar_tensor_tensor(
                            out=ydT[:, oc, :], in0=po[:], scalar=dbo_s[:, oc : oc + 1],
                            in1=clsF[:, oc, :], op0=ALU.add, op1=ALU.add,
                        )
                        dsq = dlgp.tile([128, B], F32R, tag="dsq", bufs=2)
                        nc.vector.tensor_tensor(
                            out=dsq[:], in0=ydT[:, oc, :], in1=ydT[:, oc, :],
                            op=ALU.mult,
                        )
                        nc.tensor.matmul(
                            lnpd_m[:], ones_fr[:, 0:1], ydT[:, oc, :],
                            start=(oc == 0), stop=(oc == HC - 1),
                            skip_group_check=True,
                        )
                        nc.tensor.matmul(
                            lnpd_q[:], ones_fr[:, 0:1], dsq[:],
                            start=(oc == 0), stop=(oc == HC - 1),
                            skip_group_check=True,
                        )
                        yield
                    rstd_rep, msc_rep = _ln_reps(
                        nc, rows, lnpd_m, lnpd_q, eps_t, B, HID, oscale=WSC
                    )
                    for oc in range(HC):
                        nc.vector.tensor_tensor(
                            out=x2clsT[:, oc, :], in0=ydT[:, oc, :], in1=rstd_rep[:],
                            op=ALU.mult,
                        )
                        nc.vector.tensor_tensor(
                            out=x2clsT[:, oc, :], in0=x2clsT[:, oc, :], in1=msc_rep[:],
                            op=ALU.subtract,
                        )
                        if oc < 4:
                            nc.vector.tensor_copy(
                                out=x2q[:, oc, :], in_=x2clsT[:, oc, :]
                            )
                        if oc % 2 == 1:
                            yield

                x2clsT = dlgp.tile([128, HC, B], BF)
                x2q = dlgp.tile([128, 4, B], F8)

                _drive(ffn_seq(0, None))
                _drive(ffn_seq(1, None))
                _drive(ffn_seq(2, None), dialog_stage())
                # last seq's FFN with the CLS fixup riding along
                _drive(ffn_seq(SPC - 1, x2clsT))


def _build():
    nc = bacc.Bacc(
        "TRN2", target_bir_lowering=False, debug=False, num_devices=NCORES
    )
    d = {}
    d["xbf"] = nc.dram_tensor("xbf", [SPC, 128, HC, S], BF, kind="ExternalInput")[:]
    d["x8"] = nc.dram_tensor("x8", [SPC, 128, HC, S], F8, kind="ExternalInput")[:]
    for nm in ["dwq", "dwk", "dwv", "dwo"]:
        d[nm] = nc.dram_tensor(nm, [128, HC, HID], BF, kind="ExternalInput")[:]
    for nm in ["wq_hi", "wk_hi", "wv_hi", "wao_hi"]:
        d[nm] = nc.dram_tensor(nm, [128, HC, HID], F8, kind="ExternalInput")[:]
    for nm in ["bq", "bk", "bao", "dbq", "dbk", "dbo", "bo2"]:
        d[nm] = nc.dram_tensor(nm, [128, HC], F32, kind="ExternalInput")[:]
    d["bv_rep"] = nc.dram_tensor("bv_rep", [128, HID], BF, kind="ExternalInput")[:]
    d["dbv_rep"] = nc.dram_tensor("dbv_rep", [128, HID], BF, kind="ExternalInput")[:]
    d["bi"] = nc.dram_tensor("bi", [128, IC], F32, kind="ExternalInput")[:]
    d["wi8"] = nc.dram_tensor("wi8", [128, IC, 4, 128], F8, kind="ExternalInput")[:]
    d["wib"] = nc.dram_tensor("wib", [128, IC, 2, 128], BF, kind="ExternalInput")[:]
    d["wo2_hi"] = nc.dram_tensor(
        "wo2_hi", [128, HC, IC, 128], F8, kind="ExternalInput"
    )[:]
    d["cmaskt"] = nc.dram_tensor("cmaskt", [B, NH, B], F32, kind="ExternalInput")[:]
    d["out"] = nc.dram_tensor("out", [SPC, 128, HC, S], F32, kind="ExternalOutput")[:]

    with tile.TileContext(nc, num_cores=NCORES) as tc:
        _emit(tc, d)
    nc.compile()
    return nc


def _np_bf16():
    import ml_dtypes

    return ml_dtypes.bfloat16


def _np_fp8():
    import ml_dtypes

    return ml_dtypes.float8_e4m3


def _pack_w(w):
    BF_NP = _np_bf16()
    return np.ascontiguousarray(
        np.asarray(w, np.float32).reshape(HC, 128, HID).transpose(1, 0, 2)
    ).astype(BF_NP)


def _pack_b(b, nch=HC):
    return np.ascontiguousarray(np.asarray(b, np.float32).reshape(nch, 128).T)


def _make_cmaskt():
    # additive mask, transposed [key, query], replicated per head.
    # -30 stands in for the reference's -10000 (exp(-30) ~ 9e-14 is
    # negligible next to any unmasked term, and row 0 - where every
    # in-dialog entry is masked - still reduces to softmax(s) exactly);
    # cross-dialog pairs use -60 so they stay negligible even against
    # fully-masked rows.
    pos = np.arange(TURNS)
    base = (pos[None, :] >= pos[:, None]).astype(np.float32) * (-30.0)
    cm = np.full((B, B), -60.0, np.float32)
    for dd in range(NDLG):
        cm[dd * TURNS : (dd + 1) * TURNS, dd * TURNS : (dd + 1) * TURNS] = base
    cmt = cm.T  # [key, query]
    return np.ascontiguousarray(np.tile(cmt[:, None, :], (1, NH, 1)))


_NC = None


def _get_nc():
    global _NC
    if _NC is None:
        _NC = _build()
    return _NC


def _pack_w_hilo(w):
    """fp8 hi/lo split of WSC*w in the [128, HC, HID] stationary layout."""
    FP8 = _np_fp8()
    wf = np.ascontiguousarray(
        WSC * np.asarray(w, np.float32).reshape(HC, 128, HID).transpose(1, 0, 2)
    )
    hi = wf.astype(FP8)
    lo = (wf - hi.astype(np.float32)).astype(FP8)
    return hi, lo


def _prepare_in_maps(inputs):
    BF_NP = _np_bf16()
    FP8_NP = _np_fp8()
    f = lambda k: np.asarray(inputs[k], np.float32)
    shared = {
        "dwq": _pack_w(f("dWq")),
        "dwk": _pack_w(f("dWk")),
        "dwv": _pack_w(f("dWv")),
        "dwo": _pack_w(f("dWo")),
        "bq": _pack_b(WSC * f("bq")),
        "bk": _pack_b(WSC * f("bk")),
        "bao": _pack_b(WSC * WSC * f("bao")),
        "dbq": _pack_b(WSC * f("dbq")),
        "dbk": _pack_b(WSC * f("dbk")),
        "dbo": _pack_b(WSC * f("dbo")),
        "bo2": _pack_b(WSC * f("bo2")),
        "bv_rep": np.ascontiguousarray(
            np.tile(WSC * f("bv").reshape(1, HID), (128, 1))
        ).astype(BF_NP),
        "dbv_rep": np.ascontiguousarray(
            np.tile(WSC * f("dbv").reshape(1, HID), (128, 1))
        ).astype(BF_NP),
        "bi": _pack_b(f("bi"), IC),
        "cmaskt": _make_cmaskt(),
    }
    wif = WSC * f("Wi").reshape(HC, 128, IC, 128).transpose(1, 2, 0, 3)
    shared["wi8"] = np.ascontiguousarray(wif[:, :, 0:4]).astype(FP8_NP)
    shared["wib"] = np.ascontiguousarray(wif[:, :, 4:6]).astype(BF_NP)
    for nm, key in [("wq", "Wq"), ("wk", "Wk"), ("wv", "Wv"), ("wao", "Wao")]:
        shared[nm + "_hi"] = _pack_w_hilo(f(key))[0]
    wo2f = np.ascontiguousarray(
        WSC * f("Wo2").reshape(IC, 128, HC, 128).transpose(1, 2, 0, 3)
    )
    shared["wo2_hi"] = wo2f.astype(FP8_NP)
    x = np.asarray(inputs["hidden_states"], np.float32)
    in_maps = []
    for c in range(NCORES):
        xs = x[c * SPC : (c + 1) * SPC]  # [4, 512, 768]
        xp = np.ascontiguousarray(
            xs.transpose(0, 2, 1).reshape(SPC, HC, 128, S).transpose(0, 2, 1, 3)
        )
        in_maps.append(
            {
                **shared,
                # xbf carries WSC^2 (the AO residual add matches the
                # WSC^2-scaled attention-output psum); x8 is the unscaled
                # fp8 GEMM operand.
                "xbf": (WSC * WSC * xp).astype(BF_NP),
                "x8": xp.astype(FP8_NP),
            }
        )
    return in_maps


def _assemble(results):
    parts = []
    for c in range(NCORES):
        o = np.asarray(results[c]["out"], np.float32)  # [4, 128, 6, 512]
        parts.append(o.transpose(0, 2, 1, 3).reshape(SPC, HID, S).transpose(0, 2, 1))
    return np.ascontiguousarray(np.concatenate(parts, axis=0))


def run(inputs, trace=False):
    nc = _get_nc()
    in_maps = _prepare_in_maps(inputs)
    res = run_bass_kernel_spmd(
        nc, in_maps, core_ids=list(range(NCORES)), trace=trace
    )
    return _assemble(res.results), res


def kernel(**inputs):
    out, _ = run(inputs)
    return out


# revision 84
# speedup vs baseline: 1.0083x; 1.0083x over previous
"""Trainium2 Bass kernel for nn_BertLayer_47339129536519.

BertLayer with hierarchical dialog attention:
  1) token-level MHA + SelfOutput(LN)       [B=32, S=512, H=768, 12 heads]
  2) dialog attention over per-turn CLS tokens (4 dialogs x 8 turns)
  3) FFN (gelu-erf) + output LN

Sharding: data-parallel over the 32 sequences, 4 per core on 8 cores.
The dialog attention mixes CLS vectors across cores -> tiny AllGather
(32x768) and every core redundantly computes the (tiny) dialog block.

v9 vs v2 (the 885us baseline):
  * the chip runs power-throttled (avg tensor-util limit ~70%), so
    wall time ~ PE-busy / 0.7: every tensor-engine cycle cut pays 1.4x.
  * fp8e4 DoubleRow matmuls (2 k-tiles per pass ~ 2x bf16 FLOPs) for
    the V/Q/K/AO projections (plain fp8: softmax+LN+residual damp the
    quantization to ~1e-3 of the output), for Wo2 (fp8 interT), and
    for 4 of Wi's 6 k-chunks (hybrid: error scales sqrt(2/3)).
    Weights pre-scaled x32 into e4m3's normal range; the scale rides
    psum and is folded into gelu/exp input scales and LN scale
    invariance (x carries x1024 via the host, x1 carries x32).
  * phase 1 software-pipelined: projections of seq s+1 interleave with
    attention (scores/exp/PV/AO/LN) of seq s via weighted round-robin
    generator emission, keeping the PE fed while ACT chews exp.
  * LayerNorm rstd: ACT Sqrt + reciprocal_approx_fast + one Newton
    step (the raw approx's ~4e-3 rstd error scales the output 1:1).
  * dialog attention emitted interleaved with FFN(seq2) chunks so its
    skinny dependency chain doesn't head-of-line-block the PE queue;
    CLS fixup rides FFN(seq3) (stale-CLS trick for columns 1..S-1).
  * qt bias + softmax denominator copies on DVE (ACT is exp-bound).
"""

import numpy as np

import concourse.bass as bass
import concourse.mybir as mybir
import concourse.tile as tile
from concourse import bacc
from concourse.bass_utils import run_bass_kernel_spmd

HID, NH, HD, S = 768, 12, 64, 512
B, NCORES, SPC = 32, 8, 4  # batch, cores, sequences per core
TURNS = 8
NDLG = B // TURNS  # 4 dialogs
HC = HID // 128  # 6 hidden-dim chunks of 128
IC = (4 * HID) // 128  # 24 intermediate chunks
INTER = 4 * HID  # 3072
EPS = 1e-12
ISCALE = 0.125  # 1/sqrt(64)

F32 = mybir.dt.float32
F32R = mybir.dt.float32r
BF = mybir.dt.bfloat16
F8 = mybir.dt.float8e4
DR = mybir.MatmulPerfMode.DoubleRow
AF = mybir.ActivationFunctionType
ALU = mybir.AluOpType
AX = mybir.AxisListType
WSC = 32.0  # fp8 weight pre-scale (0.02-sigma weights -> normal e4m3 range)


def _drive(*gens, weights=None):
    """Weighted round-robin drive: interleaves generator emission so
    independent work lands between dependent chains in each engine's
    (in-order) queue.  weights[i] = how many steps of gens[i] per cycle
    (fractional allowed: 0.5 = one step every other cycle)."""
    live = [(g, (weights[i] if weights else 1.0))
            for i, g in enumerate(gens) if g is not None]
    credit = [0.0] * len(live)
    while live:
        for i, (g, w) in enumerate(list(live)):
            if g is None:
                continue
            credit[i] += w
            while credit[i] >= 1.0 and g is not None:
                credit[i] -= 1.0
                try:
                    next(g)
                except StopIteration:
                    live[i] = (None, w)
                    g = None
        if all(g is None for g, _ in live):
            break


def _ln_reps(nc, rows, lnp_m, lnp_q, eps_t, n, dim, oscale=1.0):
    """From accumulated sum (lnp_m[1,n]) / sum-of-squares (lnp_q[1,n]) psum
    rows, produce broadcast [128, n] tiles (rstd_rep, mscaled_rep) so that
    oscale*normalized = y * rstd_rep - mscaled_rep.  The reciprocal is
    approx_fast + one Newton step (error ~(4e-3)^2, vs 4e-3 for the raw
    approx, which directly scales the LN output).  oscale folds into the
    Newton bracket for free.  LN is scale-invariant in y, so callers can
    feed pre-scaled y without adjusting anything here."""
    # scratch rows packed on partitions of one tile: [1,n] tiles cost a
    # full 2KB of per-partition address space each.  mean/rstd stay
    # partition-0 tiles (partition_broadcast reads partition 0).
    # (two SBUF inputs of a DVE op must share base partition -> keep all
    #  row tiles at partition 0; fold intermediates in place)
    mean = rows.tile([1, n], F32, tag="ln_mean", bufs=1)
    nc.vector.tensor_scalar_mul(mean[:], lnp_m[:], 1.0 / dim)
    rstd = rows.tile([1, n], F32, tag="ln_rstd", bufs=1)
    # rstd holds mean^2 transiently
    nc.vector.tensor_tensor(out=rstd[:], in0=mean[:], in1=mean[:], op=ALU.mult)
    var = rows.tile([1, n], F32, tag="ln_var", bufs=1)
    nc.vector.scalar_tensor_tensor(
        out=var[:], in0=lnp_q[:], scalar=1.0 / dim, in1=rstd[:],
        op0=ALU.mult, op1=ALU.subtract,
    )
    nc.scalar.activation(var[:], var[:], AF.Sqrt, bias=eps_t[:])
    r0 = rows.tile([1, n], F32, tag="ln_r0", bufs=1)
    nc.vector.reciprocal_approx_fast(out=r0[:], in_=var[:])
    # Newton: rstd = r0 * (2 - var * r0); var becomes the bracket in place
    nc.vector.tensor_tensor(out=var[:], in0=var[:], in1=r0[:], op=ALU.mult)
    nc.vector.tensor_scalar(
        out=var[:], in0=var[:], scalar1=-oscale, scalar2=2.0 * oscale,
        op0=ALU.mult, op1=ALU.add,
    )
    nc.vector.tensor_tensor(out=rstd[:], in0=r0[:], in1=var[:], op=ALU.mult)
    nc.vector.tensor_tensor(out=mean[:], in0=mean[:], in1=rstd[:], op=ALU.mult)
    rstd_rep = rows.tile([128, n], F32, tag="ln_rstd_rep", bufs=1)
    nc.gpsimd.partition_broadcast(rstd_rep[:], rstd[:])
    msc_rep = rows.tile([128, n], F32, tag="ln_msc_rep", bufs=1)
    nc.gpsimd.partition_broadcast(msc_rep[:], mean[:])
    return rstd_rep, msc_rep


def _emit(tc, d):
    nc = tc.nc
    from concourse import library_config

    nc.gpsimd.load_library(library_config.attn)  # for partition_broadcast

    with (
        tc.tile_pool(name="setup", bufs=1) as setup,
        tc.tile_pool(name="rows", bufs=2) as rows,
        tc.tile_pool(name="dram", bufs=1, space="DRAM") as dram,
    ):
        # ---- constants / small params ----
        ones_f32 = setup.tile([128, 2], F32)
        nc.vector.memset(ones_f32, 1.0)
        ones_fr = ones_f32.bitcast(F32R)
        ones_bf = setup.tile([128, 2], BF)
        nc.vector.memset(ones_bf, 1.0)
        eps_t = setup.tile([1, 1], F32)
        nc.vector.memset(eps_t, EPS)

        def load_small(name, dt=F32):
            t = setup.tile(list(d[name].shape), dt, name="sb_" + name)
            nc.sync.dma_start(t[:], d[name][:])
            return t

        bq_s = load_small("bq")
        bk_s = load_small("bk")
        bao_s = load_small("bao")
        bv_rep = load_small("bv_rep", BF)
        dbq_s = load_small("dbq")
        dbk_s = load_small("dbk")
        dbo_s = load_small("dbo")
        bi_s = load_small("bi")
        bo2_s = load_small("bo2")

        # persistent-through-kernel tiles.  x1 holds 32*LN1out (bf16 is
        # scale-free; LN2 washes the factor out).  x1q: fp8 copy of hid
        # chunks 0-3 for the hybrid-precision Wi GEMM (4 chunks fp8
        # DoubleRow + 2 chunks bf16 -> 2/3 of the x/W quantization noise
        # at 2/3 of the bf16 cycles).
        x1 = setup.tile([128, SPC, HC, S], BF)
        x1q = setup.tile([128, SPC, 4, S], F8)
        cls_in = dram.tile([128, HC, SPC], F32, name="cls_in")
        cls_out = dram.tile([NCORES * 128, HC, SPC], F32, name="cls_out")

        # FFN Wi weights: resident, DMA overlapped with phase 1
        with tc.tile_pool(name="ffw", bufs=1) as ffw:
          wi8_s = ffw.tile([128, IC, 4, 128], F8)
          wib_s = ffw.tile([128, IC, 2, 128], BF)
          # ========================= PHASE 1: token attention =================
          with (
            tc.tile_pool(name="attw", bufs=1) as attw,
            tc.tile_pool(name="attp", bufs=1) as attp,
            tc.tile_pool(name="psP", bufs=2, space="PSUM") as psP,
            tc.tile_pool(name="psS", bufs=2, space="PSUM") as psS,
            tc.tile_pool(name="psC", bufs=2, space="PSUM") as psC,
            tc.tile_pool(name="psL", bufs=1, space="PSUM") as psL,
          ):
            # attention weights in plain fp8 (WSC-scaled): softmax + LN +
            # residual damp the quantization to ~1e-3 of the output, and
            # the DoubleRow matmuls run the projections at 2x bf16 rate.
            attws = {}
            for nm in ["wv", "wq", "wk", "wao"]:
                t = attw.tile([128, HC, HID], F8, name="sb_" + nm)
                nc.sync.dma_start(t[:], d[nm + "_hi"][:])
                attws[nm] = t
            for g in range(8):
                nc.sync.dma_start(
                    wi8_s[:, g * 3 : (g + 1) * 3, :, :],
                    d["wi8"][:, g * 3 : (g + 1) * 3],
                )
                nc.sync.dma_start(
                    wib_s[:, g * 3 : (g + 1) * 3, :, :],
                    d["wib"][:, g * 3 : (g + 1) * 3],
                )

            xtiles = {}

            def load_x(si):
                if si >= SPC:
                    return
                t = attp.tile([128, HC, S], BF, tag="xs", bufs=2, name="xs")
                for g in range(3):
                    nc.sync.dma_start(
                        t[:, 2 * g : 2 * g + 2, :], d["xbf"][si, :, 2 * g : 2 * g + 2]
                    )
                t8 = attp.tile([128, HC, S], F8, tag="xs8", bufs=2, name="xs8")
                nc.sync.dma_start(t8[:], d["x8"][si])
                xtiles[si] = (t, t8)

            # per-seq projection outputs (2 generations live: s and s+1)
            vtiles, qtiles, ktiles = {}, {}, {}

            def proj_stage(s):
                """V/Q/K projections for seq s: fp8 DoubleRow over hc pairs,
                hi then lo weight halves accumulating into one psum chain.
                psum comes out at WSC*(x@W); the bias evictions rescale."""
                xs, xs8 = xtiles[s]
                v_aug = attp.tile([128, 4, NH, HD + 1], BF, tag="vaug", bufs=2)
                nc.vector.memset(v_aug[:, :, :, HD : HD + 1], 1.0)
                vtiles[s] = v_aug
                GH = HC // 2
                wv8 = attws["wv"]
                for sc in range(4):
                    for half in range(2):
                        pv = psP.tile([128, S], F32, tag="psP")
                        for g in range(GH):
                            nc.tensor.matmul(
                                pv[:, :384],
                                xs8[:, 2 * g : 2 * g + 2,
                                    sc * 128 : (sc + 1) * 128],
                                wv8[:, 2 * g : 2 * g + 2,
                                    half * 384 : (half + 1) * 384],
                                start=(g == 0),
                                stop=(g == GH - 1),
                                perf_mode=DR,
                            )
                        nc.vector.tensor_tensor(
                            out=v_aug[:, sc, half * 6 : half * 6 + 6, 0:HD],
                            in0=pv[:, :384].rearrange("p (h e) -> p h e", e=HD),
                            in1=bv_rep[
                                :, half * 384 : (half + 1) * 384
                            ].rearrange("p (h e) -> p h e", e=HD),
                            op=ALU.add,
                        )
                        yield
                qt = attp.tile([128, HC, S], F8, tag="qt", bufs=2)
                kt = attp.tile([128, HC, S], F8, tag="kt", bufs=2)
                qtiles[s], ktiles[s] = qt, kt
                wq8, wk8 = attws["wq"], attws["wk"]
                for dc in range(HC):
                    pq = psP.tile([128, S], F32, tag="psP")
                    for g in range(GH):
                        nc.tensor.matmul(
                            pq[:],
                            wq8[:, 2 * g : 2 * g + 2, dc * 128 : (dc + 1) * 128],
                            xs8[:, 2 * g : 2 * g + 2, :],
                            start=(g == 0),
                            stop=(g == GH - 1),
                            perf_mode=DR,
                        )
                    nc.vector.tensor_scalar_add(
                        out=qt[:, dc, :], in0=pq[:], scalar1=bq_s[:, dc : dc + 1]
                    )
                    yield
                    pk = psP.tile([128, S], F32, tag="psP")
                    for g in range(GH):
                        nc.tensor.matmul(
                            pk[:],
                            wk8[:, 2 * g : 2 * g + 2, dc * 128 : (dc + 1) * 128],
                            xs8[:, 2 * g : 2 * g + 2, :],
                            start=(g == 0),
                            stop=(g == GH - 1),
                            perf_mode=DR,
                        )
                    nc.vector.tensor_scalar_add(
                        out=kt[:, dc, :], in0=pk[:], scalar1=bk_s[:, dc : dc + 1]
                    )
                    yield

            def attn_stage(s):
                """scores/softmax/PV/AO/LN1 for seq s (needs proj_stage(s)
                complete). ACT(exp)-dominated; meant to overlap
                proj_stage(s+1) on the PE."""
                xs, _xs8 = xtiles.pop(s)
                v_aug = vtiles.pop(s)
                qt = qtiles.pop(s)
                kt = ktiles.pop(s)
                load_x(s + 2)

                # ctxT carries WSC*ctx (v_aug holds WSC*v with a 1.0 ones
                # column, so the PV ratio comes out WSC-scaled), fp8 for
                # the DoubleRow AO projection.
                ctxT = attp.tile([128, HC, S], F8, tag="ctxT", bufs=1)

                def emit_scores(dc):
                    probsT = attp.tile(
                        [128, 4, 2, S], BF, tag="probsT", bufs=2, name="probsT"
                    )
                    for kc in range(4):
                        pse = psS.tile([128, S], F32, tag="ps_s")
                        pso = psS.tile([128, S], F32, tag="ps_s")
                        nc.tensor.matmul(
                            pse[:],
                            kt[0:64, dc, kc * 128 : (kc + 1) * 128],
                            qt[0:64, dc, :],
                            start=True, stop=True,
                        )
                        nc.tensor.matmul(
                            pso[:],
                            kt[64:128, dc, kc * 128 : (kc + 1) * 128],
                            qt[64:128, dc, :],
                            start=True, stop=True,
                        )
                        nc.scalar.activation(
                            probsT[:, kc, 0, :], pse[:], AF.Exp,
                            scale=ISCALE / (WSC * WSC),
                        )
                        nc.scalar.activation(
                            probsT[:, kc, 1, :], pso[:], AF.Exp,
                            scale=ISCALE / (WSC * WSC),
                        )
                    return probsT

                def emit_pv(dc, probsT, eo):
                    h = 2 * dc + eo
                    pc = psC.tile([HD + 1, S], F32, tag="pc")
                    for kc in range(4):
                        nc.tensor.matmul(
                            pc[:],
                            v_aug[:, kc, h, :],
                            probsT[:, kc, eo, :],
                            start=(kc == 0),
                            stop=(kc == 3),
                        )
                    dn = rows.tile([1, S], F32, tag="dn", bufs=2)
                    nc.vector.tensor_copy(out=dn[:], in_=pc[HD : HD + 1, :])
                    rcp = rows.tile([1, S], F32, tag="rcp", bufs=2)
                    nc.vector.reciprocal_approx_fast(out=rcp[:], in_=dn[:])
                    rep = rows.tile([HD, S], F32, tag="rep")
                    nc.gpsimd.partition_broadcast(rep[:], rcp[:])
                    nc.vector.tensor_tensor(
                        out=ctxT[eo * 64 : eo * 64 + 64, dc, :],
                        in0=pc[0:HD, :],
                        in1=rep[:],
                        op=ALU.mult,
                    )

                prev = None
                for dc in range(HC):
                    pt = emit_scores(dc)
                    yield
                    if prev is not None:
                        emit_pv(prev[0], prev[1], 0)
                        yield
                        emit_pv(prev[0], prev[1], 1)
                        yield
                    prev = (dc, pt)
                emit_pv(prev[0], prev[1], 0)
                yield
                emit_pv(prev[0], prev[1], 1)
                yield

                # ---- attention output projection + residual + LN1 ----
                y = attp.tile([128, HC, S], F32R, tag="y")
                lnp_m = psL.tile([1, S], F32, tag="lnm")
                lnp_q = psL.tile([1, S], F32, tag="lnq")
                GH = HC // 2
                wao8 = attws["wao"]
                for dc in range(HC):
                    pa = psP.tile([128, S], F32, tag="psP")
                    for g in range(GH):
                        nc.tensor.matmul(
                            pa[:],
                            wao8[:, 2 * g : 2 * g + 2, dc * 128 : (dc + 1) * 128],
                            ctxT[:, 2 * g : 2 * g + 2, :],
                            start=(g == 0),
                            stop=(g == GH - 1),
                            perf_mode=DR,
                        )
                    nc.vector.scalar_tensor_tensor(
                        out=y[:, dc, :], in0=pa[:], scalar=bao_s[:, dc : dc + 1],
                        in1=xs[:, dc, :], op0=ALU.add, op1=ALU.add,
                    )
                    sq = attp.tile([128, S], BF, tag="sq", bufs=1)
                    nc.vector.tensor_tensor(
                        out=sq[:], in0=y[:, dc, :], in1=y[:, dc, :], op=ALU.mult
                    )
                    nc.tensor.matmul(
                        lnp_m[:], ones_fr[:, 0:1], y[:, dc, :],
                        start=(dc == 0), stop=(dc == HC - 1),
                        skip_group_check=True,
                    )
                    nc.tensor.matmul(
                        lnp_q[:], ones_bf[:, 0:1], sq[:],
                        start=(dc == 0), stop=(dc == HC - 1),
                        skip_group_check=True,
                    )
                    yield
                rstd_rep, msc_rep = _ln_reps(
                    nc, rows, lnp_m, lnp_q, eps_t, S, HID, oscale=WSC
                )
                for dc in range(HC):
                    nc.vector.tensor_tensor(
                        out=x1[:, s, dc, :], in0=y[:, dc, :], in1=rstd_rep[:],
                        op=ALU.mult,
                    )
                    nc.vector.tensor_tensor(
                        out=x1[:, s, dc, :], in0=x1[:, s, dc, :], in1=msc_rep[:],
                        op=ALU.subtract,
                    )
                    if dc < 4:
                        nc.vector.tensor_copy(
                            out=x1q[:, s, dc, :], in_=x1[:, s, dc, :]
                        )
                    yield
                clsst = rows.tile([128, HC, 1], F32, tag="clsst")
                nc.vector.tensor_copy(out=clsst[:], in_=x1[:, s, :, 0:1])
                nc.sync.dma_start(cls_in[:, :, s : s + 1], clsst[:])

            load_x(0)
            load_x(1)
            _drive(proj_stage(0))
            for s in range(SPC):
                _drive(
                    attn_stage(s),
                    proj_stage(s + 1) if s + 1 < SPC else None,
                )

          # ========= PHASE 2+3: dialog attention overlapped with FFN =========
            #
            # FFN for all 4 seqs runs on x1 with the *stale* CLS column; every
            # token's FFN+LN2 is independent, so only column 0 of each output
            # is affected - and column 0 is not stored from the main pass.
            # The dialog block (gather + tiny attention) is emitted interleaved
            # with seq 2's FFN chunks, and a CLS fixup pass (all 32 updated
            # CLS vectors, N=32 moving) rides along seq 3's FFN loops.
          with (
                tc.tile_pool(name="dlgw", bufs=1) as dlgw,
                tc.tile_pool(name="dlgp", bufs=1) as dlgp,
                tc.tile_pool(name="ffp", bufs=1) as ffp,
                tc.tile_pool(name="psZ", bufs=2, space="PSUM") as psZ,
                tc.tile_pool(name="psO", bufs=2, space="PSUM") as psO,
                tc.tile_pool(name="psL2", bufs=1, space="PSUM") as psL2,
                tc.tile_pool(name="psD", bufs=1, space="PSUM") as psD,
          ):
                nc.gpsimd.collective_compute(
                    "AllGather",
                    ALU.bypass,
                    replica_groups=[list(range(NCORES))],
                    ins=[cls_in.opt()],
                    outs=[cls_out.opt()],
                )
                dwq_s = dlgw.tile([128, HC, HID], BF, tag="dw", bufs=2)
                nc.sync.dma_start(dwq_s[:], d["dwq"][:])
                dwk_s = dlgw.tile([128, HC, HID], BF, tag="dw", bufs=2)
                nc.sync.dma_start(dwk_s[:], d["dwk"][:])
                dbv_rep = dlgw.tile([128, HID], BF, name="sb_dbv_rep")
                nc.sync.dma_start(dbv_rep[:], d["dbv_rep"][:])
                cmaskt_s = dlgw.tile([B, NH, B], F32, name="sb_cmaskt")
                nc.sync.dma_start(cmaskt_s[:], d["cmaskt"][:])

                def ffn_seq(s, fix):
                    x2 = fix  # x2clsT (bf16, WSC-scaled) when fix is set
                    # Wi runs in bf16 (same cycles as a split-fp8 pair, but
                    # no x/W quantization error); psum = WSC*(x@Wi) since x1
                    # carries WSC -> fold 1/WSC into the gelu input scale.
                    # Wo2 runs fp8 DoubleRow with hi+lo split weights; only
                    # the fp8 interT quantization error remains.
                    interT = ffp.tile([128, IC, S], F8, tag="interT")
                    if fix:
                        interC = ffp.tile([128, IC, B], F8, tag="interC")
                    for ic in range(IC):
                        pz = psZ.tile([128, S], F32, tag="pz")
                        if fix:
                            pzc = psZ.tile([128, B], F32, tag="pz")
                        for g in range(2):
                            nc.tensor.matmul(
                                pz[:], wi8_s[:, ic, 2 * g : 2 * g + 2, :],
                                x1q[:, s, 2 * g : 2 * g + 2, :],
                                start=(g == 0), stop=False, perf_mode=DR,
                            )
                            if fix:
                                nc.tensor.matmul(
                                    pzc[:], wi8_s[:, ic, 2 * g : 2 * g + 2, :],
                                    x2q[:, 2 * g : 2 * g + 2, :],
                                    start=(g == 0), stop=False, perf_mode=DR,
                                )
                        for hb in range(2):
                            nc.tensor.matmul(
                                pz[:], wib_s[:, ic, hb, :], x1[:, s, 4 + hb, :],
                                start=False, stop=(hb == 1),
                            )
                            if fix:
                                nc.tensor.matmul(
                                    pzc[:], wib_s[:, ic, hb, :], x2[:, 4 + hb, :],
                                    start=False, stop=(hb == 1),
                                )
                        nc.scalar.activation(
                            interT[:, ic, :], pz[:], AF.Gelu,
                            bias=bi_s[:, ic : ic + 1], scale=1.0 / (WSC * WSC),
                        )
                        if fix:
                            nc.scalar.activation(
                                interC[:, ic, :], pzc[:], AF.Gelu,
                                bias=bi_s[:, ic : ic + 1],
                                scale=1.0 / (WSC * WSC),
                            )
                        if ic % 2 == 1:
                            yield
                    y2 = ffp.tile([128, HC, S], F32R, tag="y2")
                    lnp2_m = psL2.tile([1, S], F32, tag="lnm")
                    lnp2_q = psL2.tile([1, S], F32, tag="lnq")
                    if fix:
                        y2c = ffp.tile([128, HC, B], F32R, tag="y2c")
                        lnc_m = psD.tile([1, B], F32, tag="pd")
                        lnc_q = psD.tile([1, B], F32, tag="pdo")
                    GI = IC // 2
                    for oc in range(HC):
                        wo2_sl = ffp.tile([128, IC, 128], F8, tag="wo2", bufs=2)
                        for g in range(3):
                            nc.sync.dma_start(
                                wo2_sl[:, 8 * g : 8 * g + 8, :],
                                d["wo2_hi"][:, oc, 8 * g : 8 * g + 8],
                            )
                        po = psO.tile([128, S], F32, tag="po")
                        if fix:
                            poc = psO.tile([128, B], F32, tag="po")
                        for g in range(GI):
                            nc.tensor.matmul(
                                po[:],
                                wo2_sl[:, 2 * g : 2 * g + 2, :],
                                interT[:, 2 * g : 2 * g + 2, :],
                                start=(g == 0),
                                stop=(g == GI - 1),
                                perf_mode=DR,
                            )
                            if fix:
                                nc.tensor.matmul(
                                    poc[:],
                                    wo2_sl[:, 2 * g : 2 * g + 2, :],
                                    interC[:, 2 * g : 2 * g + 2, :],
                                    start=(g == 0),
                                    stop=(g == GI - 1),
                                    perf_mode=DR,
                                )
                        nc.vector.scalar_tensor_tensor(
                            out=y2[:, oc, :], in0=po[:],
                            scalar=bo2_s[:, oc : oc + 1], in1=x1[:, s, oc, :],
                            op0=ALU.add, op1=ALU.add,
                        )
                        fsq = ffp.tile([128, S], F32R, tag="fsq", bufs=2)
                        nc.vector.tensor_tensor(
                            out=fsq[:], in0=y2[:, oc, :], in1=y2[:, oc, :],
                            op=ALU.mult,
                        )
                        nc.tensor.matmul(
                            lnp2_m[:], ones_fr[:, 0:1], y2[:, oc, :],
                            start=(oc == 0), stop=(oc == HC - 1),
                            skip_group_check=True,
                        )
                        nc.tensor.matmul(
                            lnp2_q[:], ones_fr[:, 0:1], fsq[:],
                            start=(oc == 0), stop=(oc == HC - 1),
                            skip_group_check=True,
                        )
                        if fix:
                            nc.vector.scalar_tensor_tensor(
                                out=y2c[:, oc, :], in0=poc[:],
                                scalar=bo2_s[:, oc : oc + 1], in1=x2[:, oc, :],
                                op0=ALU.add, op1=ALU.add,
                            )
                            fsqc = ffp.tile([128, B], F32R, tag="fsqc", bufs=2)
                            nc.vector.tensor_tensor(
                                out=fsqc[:], in0=y2c[:, oc, :], in1=y2c[:, oc, :],
                                op=ALU.mult,
                            )
                            nc.tensor.matmul(
                                lnc_m[:], ones_fr[:, 0:1], y2c[:, oc, :],
                                start=(oc == 0), stop=(oc == HC - 1),
                                skip_group_check=True,
                            )
                            nc.tensor.matmul(
                                lnc_q[:], ones_fr[:, 0:1], fsqc[:],
                                start=(oc == 0), stop=(oc == HC - 1),
                                skip_group_check=True,
                            )
                        yield
                    rstd_rep, msc_rep = _ln_reps(
                        nc, rows, lnp2_m, lnp2_q, eps_t, S, HID
                    )
                    outst = ffp.tile([128, HC, S], F32, tag="outst", bufs=2)
                    for oc in range(HC):
                        nc.vector.tensor_tensor(
                            out=outst[:, oc, :], in0=y2[:, oc, :],
                            in1=rstd_rep[:], op=ALU.mult,
                        )
                        nc.vector.tensor_tensor(
                            out=outst[:, oc, :], in0=outst[:, oc, :],
                            in1=msc_rep[:], op=ALU.subtract,
                        )
                        if oc % 2 == 1:
                            yield
                    for g in range(3):
                        nc.sync.dma_start(
                            d["out"][s][:, 2 * g : 2 * g + 2, 1:S],
                            outst[:, 2 * g : 2 * g + 2, 1:S],
                        )
                    if fix:
                        rsc, msc = _ln_reps(nc, rows, lnc_m, lnc_q, eps_t, B, HID)
                        outc = ffp.tile([128, HC, B], F32, tag="outc")
                        for oc in range(HC):
                            nc.vector.tensor_tensor(
                                out=outc[:, oc, :], in0=y2c[:, oc, :],
                                in1=rsc[:], op=ALU.mult,
                            )
                            nc.vector.tensor_tensor(
                                out=outc[:, oc, :], in0=outc[:, oc, :],
                                in1=msc[:], op=ALU.subtract,
                            )
                        outcl = ffp.tile([128, HC, 1, SPC], F32, tag="outcl")
                        pid = nc.partition_id()
                        nc.vector.tensor_copy(
                            out=outcl[:],
                            in_=outc.rearrange("p c (r s) -> p c r s", s=SPC)[
                                :, :, bass.ds(pid, 1), :
                            ],
                        )
                        for s2 in range(SPC):
                            nc.sync.dma_start(
                                d["out"][s2][:, :, 0:1], outcl[:, :, 0, s2 : s2 + 1]
                            )

                def dialog_stage():
                    # -------------------- dialog attention (tiny) ------------
                    clsF = dlgp.tile([128, HC, B], F32)
                    for r in range(NCORES):
                        nc.sync.dma_start(
                            clsF[:, :, r * SPC : (r + 1) * SPC],
                            cls_out[r * 128 : (r + 1) * 128, :, :],
                        )
                    clsT = dlgp.tile([128, HC, B], BF)
                    nc.vector.tensor_copy(out=clsT[:], in_=clsF[:])
                    yield

                    qdT = dlgp.tile([128, HC, B], BF)
                    kdT = dlgp.tile([128, HC, B], BF)
                    for dc in range(HC):
                        pq = psD.tile([128, B], F32, tag="pd")
                        for hc in range(HC):
                            nc.tensor.matmul(
                                pq[:], dwq_s[:, hc, dc * 128 : (dc + 1) * 128],
                                clsT[:, hc, :], start=(hc == 0), stop=(hc == HC - 1),
                            )
                        nc.vector.tensor_scalar_add(
                            out=qdT[:, dc, :], in0=pq[:], scalar1=dbq_s[:, dc : dc + 1]
                        )
                        yield
                        pk = psD.tile([128, B], F32, tag="pdo")
                        for hc in range(HC):
                            nc.tensor.matmul(
                                pk[:], dwk_s[:, hc, dc * 128 : (dc + 1) * 128],
                                clsT[:, hc, :], start=(hc == 0), stop=(hc == HC - 1),
                            )
                        nc.vector.tensor_scalar_add(
                            out=kdT[:, dc, :], in0=pk[:], scalar1=dbk_s[:, dc : dc + 1]
                        )
                        yield
                    dwv_s = dlgw.tile([128, HC, HID], BF, tag="dw", bufs=2)
                    nc.sync.dma_start(dwv_s[:], d["dwv"][:])
                    dwo_s = dlgw.tile([128, HC, HID], BF, tag="dw", bufs=2)
                    nc.sync.dma_start(dwo_s[:], d["dwo"][:])
                    # v natural [32, 768] + ones column per head
                    vd_aug = dlgp.tile([B, NH, HD + 1], BF)
                    nc.vector.memset(vd_aug[:, :, HD : HD + 1], 1.0)
                    for half in range(2):
                        pv = psD.tile([B, 384], F32, tag="pd")
                        for hc in range(HC):
                            nc.tensor.matmul(
                                pv[:], clsT[:, hc, :],
                                dwv_s[:, hc, half * 384 : (half + 1) * 384],
                                start=(hc == 0), stop=(hc == HC - 1),
                            )
                        nc.vector.tensor_tensor(
                            out=vd_aug[:, half * 6 : half * 6 + 6, 0:HD],
                            in0=pv[:].rearrange("p (h e) -> p h e", e=HD),
                            in1=dbv_rep[:B, half * 384 : (half + 1) * 384].rearrange(
                                "p (h e) -> p h e", e=HD
                            ),
                            op=ALU.add,
                        )
                        yield

                    # transposed scores: per-head matmuls (ping-pong between the
                    # two dialog psum banks), mask+scale on DVE, one batched exp.
                    sdt = dlgp.tile([B, NH, B], F32)
                    for h in range(NH):
                        dc, off = h // 2, (h % 2) * 64
                        pss = psD.tile([B, B], F32, tag=("pdo" if h % 2 == 0 else "pd"))
                        nc.tensor.matmul(
                            pss[:], kdT[off : off + 64, dc, :],
                            qdT[off : off + 64, dc, :], start=True, stop=True,
                        )
                        nc.vector.scalar_tensor_tensor(
                            out=sdt[:, h, :], in0=pss[:],
                            scalar=ISCALE / (WSC * WSC),
                            in1=cmaskt_s[:, h, :], op0=ALU.mult, op1=ALU.add,
                        )
                        if h % 3 == 2:
                            yield
                    probsTd = dlgp.tile([B, NH, B], BF)
                    nc.scalar.activation(probsTd[:], sdt[:], AF.Exp)
                    yield
                    ctxdT = dlgp.tile([128, HC, B], BF)
                    for h in range(NH):
                        dc, off = h // 2, (h % 2) * 64
                        pcd = psD.tile(
                            [HD + 1, B], F32, tag=("pdo" if h % 2 == 0 else "pd")
                        )
                        nc.tensor.matmul(
                            pcd[:], vd_aug[:, h, :], probsTd[:, h, :],
                            start=True, stop=True,
                        )
                        dnd = rows.tile([1, B], F32, tag="dnd", bufs=2)
                        nc.vector.tensor_copy(out=dnd[:], in_=pcd[HD : HD + 1, :])
                        rcpd = rows.tile([1, B], F32, tag="rcpd", bufs=2)
                        nc.vector.reciprocal_approx_fast(out=rcpd[:], in_=dnd[:])
                        repd = rows.tile([HD, B], F32, tag="repd", bufs=2)
                        nc.gpsimd.partition_broadcast(repd[:], rcpd[:])
                        nc.vector.tensor_tensor(
                            out=ctxdT[off : off + 64, dc, :],
                            in0=pcd[0:HD, :], in1=repd[:], op=ALU.mult,
                        )
                        if h % 3 == 2:
                            yield

                    # dialog output projection + residual + LN
                    # (po shares the psO banks with the FFN po rotation;
                    #  the LN stat accumulators take the psD ping-pong banks,
                    #  which have no other users from here to the fixup.)
                    ydT = dlgp.tile([128, HC, B], F32R)
                    lnpd_m = psD.tile([1, B], F32, tag="pd")
                    lnpd_q = psD.tile([1, B], F32, tag="pdo")
                    for oc in range(HC):
                        po = psO.tile([128, B], F32, tag="po")
                        for hc in range(HC):
                            nc.tensor.matmul(
                                po[:], dwo_s[:, hc, oc * 128 : (oc + 1) * 128],
                                ctxdT[:, hc, :], start=(hc == 0), stop=(hc == HC - 1),
                            )
                        nc.vector.scalar_tensor_tensor(
                            out=ydT[:, oc, :], in0=po[:], scalar=dbo_s[:, oc : oc + 1],
                            in1=clsF[:, oc, :], op0=ALU.add, op1=ALU.add,
                        )
                        dsq = dlgp.tile([128, B], F32R, tag="dsq", bufs=2)
                        nc.vector.tensor_tensor(
                            out=dsq[:], in0=ydT[:, oc, :], in1=ydT[:, oc, :],
                            op=ALU.mult,
                        )
                        nc.tensor.matmul(
                            lnpd_m[:], ones_fr[:, 0:1], ydT[:, oc, :],
                            start=(oc == 0), stop=(oc == HC - 1),
                            skip_group_check=True,
                        )
                        nc.tensor.matmul(
                            lnpd_q[:], ones_fr[:, 0:1], dsq[:],
                            start=(oc == 0), stop=(oc == HC - 1),
                            skip_group_check=True,
                        )
                        yield
                    rstd_rep, msc_rep = _ln_reps(
                        nc, rows, lnpd_m, lnpd_q, eps_t, B, HID, oscale=WSC
                    )
                    for oc in range(HC):
                        nc.vector.tensor_tensor(
                            out=x2clsT[:, oc, :], in0=ydT[:, oc, :], in1=rstd_rep[:],
                            op=ALU.mult,
                        )
                        nc.vector.tensor_tensor(
                            out=x2clsT[:, oc, :], in0=x2clsT[:, oc, :], in1=msc_rep[:],
                            op=ALU.subtract,
                        )
                        if oc < 4:
                            nc.vector.tensor_copy(
                                out=x2q[:, oc, :], in_=x2clsT[:, oc, :]
                            )
                        if oc % 2 == 1:
                            yield

                x2clsT = dlgp.tile([128, HC, B], BF)
                x2q = dlgp.tile([128, 4, B], F8)

                _drive(ffn_seq(0, None))
                _drive(ffn_seq(1, None))
                _drive(ffn_seq(2, None), dialog_stage())
                # last seq's FFN with the CLS fixup riding along
                _drive(ffn_seq(SPC - 1, x2clsT))


def _build():
    nc = bacc.Bacc(
        "TRN2", target_bir_lowering=False, debug=False, num_devices=NCORES
    )
    d = {}
    d["xbf"] = nc.dram_tensor("xbf", [SPC, 128, HC, S], BF, kind="ExternalInput")[:]
    d["x8"] = nc.dram_tensor("x8", [SPC, 128, HC, S], F8, kind="ExternalInput")[:]
    for nm in ["dwq", "dwk", "dwv", "dwo"]:
        d[nm] = nc.dram_tensor(nm, [128, HC, HID], BF, kind="ExternalInput")[:]
    for nm in ["wq_hi", "wk_hi", "wv_hi", "wao_hi"]:
        d[nm] = nc.dram_tensor(nm, [128, HC, HID], F8, kind="ExternalInput")[:]
    for nm in ["bq", "bk", "bao", "dbq", "dbk", "dbo", "bo2"]:
        d[nm] = nc.dram_tensor(nm, [128, HC], F32, kind="ExternalInput")[:]
    d["bv_rep"] = nc.dram_tensor("bv_rep", [128, HID], BF, kind="ExternalInput")[:]
    d["dbv_rep"] = nc.dram_tensor("dbv_rep", [128, HID], BF, kind="ExternalInput")[:]
    d["bi"] = nc.dram_tensor("bi", [128, IC], F32, kind="ExternalInput")[:]
    d["wi8"] = nc.dram_tensor("wi8", [128, IC, 4, 128], F8, kind="ExternalInput")[:]
    d["wib"] = nc.dram_tensor("wib", [128, IC, 2, 128], BF, kind="ExternalInput")[:]
    d["wo2_hi"] = nc.dram_tensor(
        "wo2_hi", [128, HC, IC, 128], F8, kind="ExternalInput"
    )[:]
    d["cmaskt"] = nc.dram_tensor("cmaskt", [B, NH, B], F32, kind="ExternalInput")[:]
    d["out"] = nc.dram_tensor("out", [SPC, 128, HC, S], F32, kind="ExternalOutput")[:]

    with tile.TileContext(nc, num_cores=NCORES) as tc:
        _emit(tc, d)
    nc.compile()
    return nc


def _np_bf16():
    import ml_dtypes

    return ml_dtypes.bfloat16


def _np_fp8():
    import ml_dtypes

    return ml_dtypes.float8_e4m3


def _pack_w(w):
    BF_NP = _np_bf16()
    return np.ascontiguousarray(
        np.asarray(w, np.float32).reshape(HC, 128, HID).transpose(1, 0, 2)
    ).astype(BF_NP)


def _pack_b(b, nch=HC):
    return np.ascontiguousarray(np.asarray(b, np.float32).reshape(nch, 128).T)


def _make_cmaskt():
    # additive mask, transposed [key, query], replicated per head.
    # -30 stands in for the reference's -10000 (exp(-30) ~ 9e-14 is
    # negligible next to any unmasked term, and row 0 - where every
    # in-dialog entry is masked - still reduces to softmax(s) exactly);
    # cross-dialog pairs use -60 so they stay negligible even against
    # fully-masked rows.
    pos = np.arange(TURNS)
    base = (pos[None, :] >= pos[:, None]).astype(np.float32) * (-30.0)
    cm = np.full((B, B), -60.0, np.float32)
    for dd in range(NDLG):
        cm[dd * TURNS : (dd + 1) * TURNS, dd * TURNS : (dd + 1) * TURNS] = base
    cmt = cm.T  # [key, query]
    return np.ascontiguousarray(np.tile(cmt[:, None, :], (1, NH, 1)))


_NC = None


def _get_nc():
    global _NC
    if _NC is None:
        _NC = _build()
    return _NC


def _pack_w_hilo(w):
    """fp8 hi/lo split of WSC*w in the [128, HC, HID] stationary layout."""
    FP8 = _np_fp8()
    wf = np.ascontiguousarray(
        WSC * np.asarray(w, np.float32).reshape(HC, 128, HID).transpose(1, 0, 2)
    )
    hi = wf.astype(FP8)
    lo = (wf - hi.astype(np.float32)).astype(FP8)
    return hi, lo


def _prepare_in_maps(inputs):
    BF_NP = _np_bf16()
    FP8_NP = _np_fp8()
    f = lambda k: np.asarray(inputs[k], np.float32)
    shared = {
        "dwq": _pack_w(f("dWq")),
        "dwk": _pack_w(f("dWk")),
        "dwv": _pack_w(f("dWv")),
        "dwo": _pack_w(f("dWo")),
        "bq": _pack_b(WSC * f("bq")),
        "bk": _pack_b(WSC * f("bk")),
        "bao": _pack_b(WSC * WSC * f("bao")),
        "dbq": _pack_b(WSC * f("dbq")),
        "dbk": _pack_b(WSC * f("dbk")),
        "dbo": _pack_b(WSC * f("dbo")),
        "bo2": _pack_b(WSC * f("bo2")),
        "bv_rep": np.ascontiguousarray(
            np.tile(WSC * f("bv").reshape(1, HID), (128, 1))
        ).astype(BF_NP),
        "dbv_rep": np.ascontiguousarray(
            np.tile(WSC * f("dbv").reshape(1, HID), (128, 1))
        ).astype(BF_NP),
        "bi": _pack_b(f("bi"), IC),
        "cmaskt": _make_cmaskt(),
    }
    wif = WSC * f("Wi").reshape(HC, 128, IC, 128).transpose(1, 2, 0, 3)
    shared["wi8"] = np.ascontiguousarray(wif[:, :, 0:4]).astype(FP8_NP)
    shared["wib"] = np.ascontiguousarray(wif[:, :, 4:6]).astype(BF_NP)
    for nm, key in [("wq", "Wq"), ("wk", "Wk"), ("wv", "Wv"), ("wao", "Wao")]:
        shared[nm + "_hi"] = _pack_w_hilo(f(key))[0]
    wo2f = np.ascontiguousarray(
        WSC * f("Wo2").reshape(IC, 128, HC, 128).transpose(1, 2, 0, 3)
    )
    shared["wo2_hi"] = wo2f.astype(FP8_NP)
    x = np.asarray(inputs["hidden_states"], np.float32)
    in_maps = []
    for c in range(NCORES):
        xs = x[c * SPC : (c + 1) * SPC]  # [4, 512, 768]
        xp = np.ascontiguousarray(
            xs.transpose(0, 2, 1).reshape(SPC, HC, 128, S).transpose(0, 2, 1, 3)
        )
        in_maps.append(
            {
                **shared,
                # xbf carries WSC^2 (the AO residual add matches the
                # WSC^2-scaled attention-output psum); x8 is the unscaled
                # fp8 GEMM operand.
                "xbf": (WSC * WSC * xp).astype(BF_NP),
                "x8": xp.astype(FP8_NP),
            }
        )
    return in_maps


def _assemble(results):
    parts = []
    for c in range(NCORES):
        o = np.asarray(results[c]["out"], np.float32)  # [4, 128, 6, 512]
        parts.append(o.transpose(0, 2, 1, 3).reshape(SPC, HID, S).transpose(0, 2, 1))
    return np.ascontiguousarray(np.concatenate(parts, axis=0))


def run(inputs, trace=False):
    nc = _get_nc()
    in_maps = _prepare_in_maps(inputs)
    res = run_bass_kernel_spmd(
        nc, in_maps, core_ids=list(range(NCORES)), trace=trace
    )
    return _assemble(res.results), res


def kernel(**inputs):
    out, _ = run(inputs)
    return out


# revision 86
# speedup vs baseline: 1.0116x; 1.0033x over previous
"""Trainium2 Bass kernel for nn_BertLayer_47339129536519.

BertLayer with hierarchical dialog attention:
  1) token-level MHA + SelfOutput(LN)       [B=32, S=512, H=768, 12 heads]
  2) dialog attention over per-turn CLS tokens (4 dialogs x 8 turns)
  3) FFN (gelu-erf) + output LN

Sharding: data-parallel over the 32 sequences, 4 per core on 8 cores.
The dialog attention mixes CLS vectors across cores -> tiny AllGather
(32x768) and every core redundantly computes the (tiny) dialog block.

v9 vs v2 (the 885us baseline):
  * the chip runs power-throttled (avg tensor-util limit ~70%), so
    wall time ~ PE-busy / 0.7: every tensor-engine cycle cut pays 1.4x.
  * fp8e4 DoubleRow matmuls (2 k-tiles per pass ~ 2x bf16 FLOPs) for
    the V/Q/K/AO projections (plain fp8: softmax+LN+residual damp the
    quantization to ~1e-3 of the output), for Wo2 (fp8 interT), and
    for 4 of Wi's 6 k-chunks (hybrid: error scales sqrt(2/3)).
    Weights pre-scaled x32 into e4m3's normal range; the scale rides
    psum and is folded into gelu/exp input scales and LN scale
    invariance (x carries x1024 via the host, x1 carries x32).
  * phase 1 software-pipelined: projections of seq s+1 interleave with
    attention (scores/exp/PV/AO/LN) of seq s via weighted round-robin
    generator emission, keeping the PE fed while ACT chews exp.
  * LayerNorm rstd: ACT Sqrt + reciprocal_approx_fast + one Newton
    step (the raw approx's ~4e-3 rstd error scales the output 1:1).
  * dialog attention emitted interleaved with FFN(seq2) chunks so its
    skinny dependency chain doesn't head-of-line-block the PE queue;
    CLS fixup rides FFN(seq3) (stale-CLS trick for columns 1..S-1).
  * qt bias + softmax denominator copies on DVE (ACT is exp-bound).
"""

import numpy as np

import concourse.bass as bass
import concourse.mybir as mybir
import concourse.tile as tile
from concourse import bacc
from concourse.bass_utils import run_bass_kernel_spmd

HID, NH, HD, S = 768, 12, 64, 512
B, NCORES, SPC = 32, 8, 4  # batch, cores, sequences per core
TURNS = 8
NDLG = B // TURNS  # 4 dialogs
HC = HID // 128  # 6 hidden-dim chunks of 128
IC = (4 * HID) // 128  # 24 intermediate chunks
INTER = 4 * HID  # 3072
EPS = 1e-12
ISCALE = 0.125  # 1/sqrt(64)

F32 = mybir.dt.float32
F32R = mybir.dt.float32r
BF = mybir.dt.bfloat16
F8 = mybir.dt.float8e4
DR = mybir.MatmulPerfMode.DoubleRow
AF = mybir.ActivationFunctionType
ALU = mybir.AluOpType
AX = mybir.AxisListType
WSC = 32.0  # fp8 weight pre-scale (0.02-sigma weights -> normal e4m3 range)


def _drive(*gens, weights=None):
    """Weighted round-robin drive: interleaves generator emission so
    independent work lands between dependent chains in each engine's
    (in-order) queue.  weights[i] = how many steps of gens[i] per cycle
    (fractional allowed: 0.5 = one step every other cycle)."""
    live = [(g, (weights[i] if weights else 1.0))
            for i, g in enumerate(gens) if g is not None]
    credit = [0.0] * len(live)
    while live:
        for i, (g, w) in enumerate(list(live)):
            if g is None:
                continue
            credit[i] += w
            while credit[i] >= 1.0 and g is not None:
                credit[i] -= 1.0
                try:
                    next(g)
                except StopIteration:
                    live[i] = (None, w)
                    g = None
        if all(g is None for g, _ in live):
            break


def _ln_reps(nc, rows, lnp_m, lnp_q, eps_t, n, dim, oscale=1.0):
    """From accumulated sum (lnp_m[1,n]) / sum-of-squares (lnp_q[1,n]) psum
    rows, produce broadcast [128, n] tiles (rstd_rep, mscaled_rep) so that
    oscale*normalized = y * rstd_rep - mscaled_rep.  The reciprocal is
    approx_fast + one Newton step (error ~(4e-3)^2, vs 4e-3 for the raw
    approx, which directly scales the LN output).  oscale folds into the
    Newton bracket for free.  LN is scale-invariant in y, so callers can
    feed pre-scaled y without adjusting anything here."""
    # scratch rows packed on partitions of one tile: [1,n] tiles cost a
    # full 2KB of per-partition address space each.  mean/rstd stay
    # partition-0 tiles (partition_broadcast reads partition 0).
    # (two SBUF inputs of a DVE op must share base partition -> keep all
    #  row tiles at partition 0; fold intermediates in place)
    mean = rows.tile([1, n], F32, tag="ln_mean", bufs=1)
    nc.vector.tensor_scalar_mul(mean[:], lnp_m[:], 1.0 / dim)
    rstd = rows.tile([1, n], F32, tag="ln_rstd", bufs=1)
    # rstd holds mean^2 transiently
    nc.vector.tensor_tensor(out=rstd[:], in0=mean[:], in1=mean[:], op=ALU.mult)
    var = rows.tile([1, n], F32, tag="ln_var", bufs=1)
    nc.vector.scalar_tensor_tensor(
        out=var[:], in0=lnp_q[:], scalar=1.0 / dim, in1=rstd[:],
        op0=ALU.mult, op1=ALU.subtract,
    )
    nc.scalar.activation(var[:], var[:], AF.Sqrt, bias=eps_t[:])
    r0 = rows.tile([1, n], F32, tag="ln_r0", bufs=1)
    nc.vector.reciprocal_approx_fast(out=r0[:], in_=var[:])
    # Newton: rstd = r0 * (2 - var * r0); var becomes the bracket in place
    nc.vector.tensor_tensor(out=var[:], in0=var[:], in1=r0[:], op=ALU.mult)
    nc.vector.tensor_scalar(
        out=var[:], in0=var[:], scalar1=-oscale, scalar2=2.0 * oscale,
        op0=ALU.mult, op1=ALU.add,
    )
    nc.vector.tensor_tensor(out=rstd[:], in0=r0[:], in1=var[:], op=ALU.mult)
    nc.vector.tensor_tensor(out=mean[:], in0=mean[:], in1=rstd[:], op=ALU.mult)
    rstd_rep = rows.tile([128, n], F32, tag="ln_rstd_rep", bufs=1)
    nc.gpsimd.partition_broadcast(rstd_rep[:], rstd[:])
    msc_rep = rows.tile([128, n], F32, tag="ln_msc_rep", bufs=1)
    nc.gpsimd.partition_broadcast(msc_rep[:], mean[:])
    return rstd_rep, msc_rep


def _emit(tc, d):
    nc = tc.nc
    from concourse import library_config

    nc.gpsimd.load_library(library_config.attn)  # for partition_broadcast

    with (
        tc.tile_pool(name="setup", bufs=1) as setup,
        tc.tile_pool(name="rows", bufs=2) as rows,
        tc.tile_pool(name="dram", bufs=1, space="DRAM") as dram,
    ):
        # ---- constants / small params ----
        ones_f32 = setup.tile([128, 2], F32)
        nc.vector.memset(ones_f32, 1.0)
        ones_fr = ones_f32.bitcast(F32R)
        ones_bf = setup.tile([128, 2], BF)
        nc.vector.memset(ones_bf, 1.0)
        eps_t = setup.tile([1, 1], F32)
        nc.vector.memset(eps_t, EPS)

        def load_small(name, dt=F32):
            t = setup.tile(list(d[name].shape), dt, name="sb_" + name)
            nc.sync.dma_start(t[:], d[name][:])
            return t

        bq_s = load_small("bq")
        bk_s = load_small("bk")
        bao_s = load_small("bao")
        bv_rep = load_small("bv_rep", BF)
        dbq_s = load_small("dbq")
        dbk_s = load_small("dbk")
        dbo_s = load_small("dbo")
        bi_s = load_small("bi")
        bo2_s = load_small("bo2")

        # persistent-through-kernel tiles.  x1 holds 32*LN1out (bf16 is
        # scale-free; LN2 washes the factor out).  x1q: fp8 copy of hid
        # chunks 0-3 for the hybrid-precision Wi GEMM (4 chunks fp8
        # DoubleRow + 2 chunks bf16 -> 2/3 of the x/W quantization noise
        # at 2/3 of the bf16 cycles).
        x1 = setup.tile([128, SPC, HC, S], BF)
        x1q = setup.tile([128, SPC, 4, S], F8)
        cls_in = dram.tile([128, HC, SPC], F32, name="cls_in")
        cls_out = dram.tile([NCORES * 128, HC, SPC], F32, name="cls_out")

        # FFN Wi weights: resident, DMA overlapped with phase 1
        with tc.tile_pool(name="ffw", bufs=1) as ffw:
          wi8_s = ffw.tile([128, IC, 4, 128], F8)
          wib_s = ffw.tile([128, IC, 2, 128], BF)
          # ========================= PHASE 1: token attention =================
          with (
            tc.tile_pool(name="attw", bufs=1) as attw,
            tc.tile_pool(name="attp", bufs=1) as attp,
            tc.tile_pool(name="psP", bufs=2, space="PSUM") as psP,
            tc.tile_pool(name="psS", bufs=2, space="PSUM") as psS,
            tc.tile_pool(name="psC", bufs=2, space="PSUM") as psC,
            tc.tile_pool(name="psL", bufs=1, space="PSUM") as psL,
          ):
            # attention weights in plain fp8 (WSC-scaled): softmax + LN +
            # residual damp the quantization to ~1e-3 of the output, and
            # the DoubleRow matmuls run the projections at 2x bf16 rate.
            attws = {}
            for nm in ["wv", "wq", "wk", "wao"]:
                t = attw.tile([128, HC, HID], F8, name="sb_" + nm)
                nc.sync.dma_start(t[:], d[nm + "_hi"][:])
                attws[nm] = t
            for g in range(8):
                nc.sync.dma_start(
                    wi8_s[:, g * 3 : (g + 1) * 3, :, :],
                    d["wi8"][:, g * 3 : (g + 1) * 3],
                )
                nc.sync.dma_start(
                    wib_s[:, g * 3 : (g + 1) * 3, :, :],
                    d["wib"][:, g * 3 : (g + 1) * 3],
                )

            xtiles = {}

            def load_x(si):
                if si >= SPC:
                    return
                t = attp.tile([128, HC, S], BF, tag="xs", bufs=2, name="xs")
                for g in range(3):
                    nc.sync.dma_start(
                        t[:, 2 * g : 2 * g + 2, :], d["xbf"][si, :, 2 * g : 2 * g + 2]
                    )
                t8 = attp.tile([128, HC, S], F8, tag="xs8", bufs=2, name="xs8")
                nc.sync.dma_start(t8[:], d["x8"][si])
                xtiles[si] = (t, t8)

            # per-seq projection outputs (2 generations live: s and s+1)
            vtiles, qtiles, ktiles = {}, {}, {}

            def proj_stage(s):
                """V/Q/K projections for seq s: fp8 DoubleRow over hc pairs,
                hi then lo weight halves accumulating into one psum chain.
                psum comes out at WSC*(x@W); the bias evictions rescale."""
                xs, xs8 = xtiles[s]
                v_aug = attp.tile([128, 4, NH, HD + 1], BF, tag="vaug", bufs=2)
                nc.vector.memset(v_aug[:, :, :, HD : HD + 1], 1.0)
                vtiles[s] = v_aug
                GH = HC // 2
                wv8 = attws["wv"]
                for sc in range(4):
                    for half in range(2):
                        pv = psP.tile([128, S], F32, tag="psP")
                        for g in range(GH):
                            nc.tensor.matmul(
                                pv[:, :384],
                                xs8[:, 2 * g : 2 * g + 2,
                                    sc * 128 : (sc + 1) * 128],
                                wv8[:, 2 * g : 2 * g + 2,
                                    half * 384 : (half + 1) * 384],
                                start=(g == 0),
                                stop=(g == GH - 1),
                                perf_mode=DR,
                            )
                        nc.vector.tensor_tensor(
                            out=v_aug[:, sc, half * 6 : half * 6 + 6, 0:HD],
                            in0=pv[:, :384].rearrange("p (h e) -> p h e", e=HD),
                            in1=bv_rep[
                                :, half * 384 : (half + 1) * 384
                            ].rearrange("p (h e) -> p h e", e=HD),
                            op=ALU.add,
                        )
                        yield
                qt = attp.tile([128, HC, S], F8, tag="qt", bufs=2)
                kt = attp.tile([128, HC, S], F8, tag="kt", bufs=2)
                qtiles[s], ktiles[s] = qt, kt
                wq8, wk8 = attws["wq"], attws["wk"]
                for dc in range(HC):
                    pq = psP.tile([128, S], F32, tag="psP")
                    for g in range(GH):
                        nc.tensor.matmul(
                            pq[:],
                            wq8[:, 2 * g : 2 * g + 2, dc * 128 : (dc + 1) * 128],
                            xs8[:, 2 * g : 2 * g + 2, :],
                            start=(g == 0),
                            stop=(g == GH - 1),
                            perf_mode=DR,
                        )
                    nc.vector.tensor_scalar_add(
                        out=qt[:, dc, :], in0=pq[:], scalar1=bq_s[:, dc : dc + 1]
                    )
                    yield
                    pk = psP.tile([128, S], F32, tag="psP")
                    for g in range(GH):
                        nc.tensor.matmul(
                            pk[:],
                            wk8[:, 2 * g : 2 * g + 2, dc * 128 : (dc + 1) * 128],
                            xs8[:, 2 * g : 2 * g + 2, :],
                            start=(g == 0),
                            stop=(g == GH - 1),
                            perf_mode=DR,
                        )
                    nc.vector.tensor_scalar_add(
                        out=kt[:, dc, :], in0=pk[:], scalar1=bk_s[:, dc : dc + 1]
                    )
                    yield

            def attn_stage(s):
                """scores/softmax/PV/AO/LN1 for seq s (needs proj_stage(s)
                complete). ACT(exp)-dominated; meant to overlap
                proj_stage(s+1) on the PE."""
                xs, _xs8 = xtiles.pop(s)
                v_aug = vtiles.pop(s)
                qt = qtiles.pop(s)
                kt = ktiles.pop(s)
                load_x(s + 2)

                # ctxT carries WSC*ctx (v_aug holds WSC*v with a 1.0 ones
                # column, so the PV ratio comes out WSC-scaled), fp8 for
                # the DoubleRow AO projection.
                ctxT = attp.tile([128, HC, S], F8, tag="ctxT", bufs=1)

                def emit_scores(dc):
                    probsT = attp.tile(
                        [128, 4, 2, S], BF, tag="probsT", bufs=2, name="probsT"
                    )
                    for kc in range(4):
                        pse = psS.tile([128, S], F32, tag="ps_s")
                        pso = psS.tile([128, S], F32, tag="ps_s")
                        nc.tensor.matmul(
                            pse[:],
                            kt[0:64, dc, kc * 128 : (kc + 1) * 128],
                            qt[0:64, dc, :],
                            start=True, stop=True,
                        )
                        nc.tensor.matmul(
                            pso[:],
                            kt[64:128, dc, kc * 128 : (kc + 1) * 128],
                            qt[64:128, dc, :],
                            start=True, stop=True,
                        )
                        nc.scalar.activation(
                            probsT[:, kc, 0, :], pse[:], AF.Exp,
                            scale=ISCALE / (WSC * WSC),
                        )
                        nc.scalar.activation(
                            probsT[:, kc, 1, :], pso[:], AF.Exp,
                            scale=ISCALE / (WSC * WSC),
                        )
                    return probsT

                def emit_pv(dc, probsT, eo):
                    h = 2 * dc + eo
                    pc = psC.tile([HD + 1, S], F32, tag="pc")
                    for kc in range(4):
                        nc.tensor.matmul(
                            pc[:],
                            v_aug[:, kc, h, :],
                            probsT[:, kc, eo, :],
                            start=(kc == 0),
                            stop=(kc == 3),
                        )
                    dn = rows.tile([1, S], F32, tag="dn", bufs=2)
                    nc.vector.tensor_copy(out=dn[:], in_=pc[HD : HD + 1, :])
                    rcp = rows.tile([1, S], F32, tag="rcp", bufs=2)
                    nc.vector.reciprocal_approx_fast(out=rcp[:], in_=dn[:])
                    rep = rows.tile([HD, S], F32, tag="rep")
                    nc.gpsimd.partition_broadcast(rep[:], rcp[:])
                    nc.vector.tensor_tensor(
                        out=ctxT[eo * 64 : eo * 64 + 64, dc, :],
                        in0=pc[0:HD, :],
                        in1=rep[:],
                        op=ALU.mult,
                    )

                prev = None
                for dc in range(HC):
                    pt = emit_scores(dc)
                    yield
                    if prev is not None:
                        emit_pv(prev[0], prev[1], 0)
                        yield
                        emit_pv(prev[0], prev[1], 1)
                        yield
                    prev = (dc, pt)
                emit_pv(prev[0], prev[1], 0)
                yield
                emit_pv(prev[0], prev[1], 1)
                yield

                # ---- attention output projection + residual + LN1 ----
                y = attp.tile([128, HC, S], F32R, tag="y")
                lnp_m = psL.tile([1, S], F32, tag="lnm")
                lnp_q = psL.tile([1, S], F32, tag="lnq")
                GH = HC // 2
                wao8 = attws["wao"]
                for dc in range(HC):
                    pa = psP.tile([128, S], F32, tag="psP")
                    for g in range(GH):
                        nc.tensor.matmul(
                            pa[:],
                            wao8[:, 2 * g : 2 * g + 2, dc * 128 : (dc + 1) * 128],
                            ctxT[:, 2 * g : 2 * g + 2, :],
                            start=(g == 0),
                            stop=(g == GH - 1),
                            perf_mode=DR,
                        )
                    nc.vector.scalar_tensor_tensor(
                        out=y[:, dc, :], in0=pa[:], scalar=bao_s[:, dc : dc + 1],
                        in1=xs[:, dc, :], op0=ALU.add, op1=ALU.add,
                    )
                    sq = attp.tile([128, S], BF, tag="sq", bufs=1)
                    nc.vector.tensor_tensor(
                        out=sq[:], in0=y[:, dc, :], in1=y[:, dc, :], op=ALU.mult
                    )
                    nc.tensor.matmul(
                        lnp_m[:], ones_fr[:, 0:1], y[:, dc, :],
                        start=(dc == 0), stop=(dc == HC - 1),
                        skip_group_check=True,
                    )
                    nc.tensor.matmul(
                        lnp_q[:], ones_bf[:, 0:1], sq[:],
                        start=(dc == 0), stop=(dc == HC - 1),
                        skip_group_check=True,
                    )
                    yield
                rstd_rep, msc_rep = _ln_reps(
                    nc, rows, lnp_m, lnp_q, eps_t, S, HID, oscale=WSC
                )
                for dc in range(HC):
                    nc.vector.tensor_tensor(
                        out=x1[:, s, dc, :], in0=y[:, dc, :], in1=rstd_rep[:],
                        op=ALU.mult,
                    )
                    nc.vector.tensor_tensor(
                        out=x1[:, s, dc, :], in0=x1[:, s, dc, :], in1=msc_rep[:],
                        op=ALU.subtract,
                    )
                    if dc < 4:
                        nc.vector.tensor_copy(
                            out=x1q[:, s, dc, :], in_=x1[:, s, dc, :]
                        )
                    yield
                clsst = rows.tile([128, HC, 1], F32, tag="clsst")
                nc.vector.tensor_copy(out=clsst[:], in_=x1[:, s, :, 0:1])
                nc.sync.dma_start(cls_in[:, :, s : s + 1], clsst[:])

            load_x(0)
            load_x(1)
            _drive(proj_stage(0))
            for s in range(SPC):
                _drive(
                    attn_stage(s),
                    proj_stage(s + 1) if s + 1 < SPC else None,
                )

          # ========= PHASE 2+3: dialog attention overlapped with FFN =========
            #
            # FFN for all 4 seqs runs on x1 with the *stale* CLS column; every
            # token's FFN+LN2 is independent, so only column 0 of each output
            # is affected - and column 0 is not stored from the main pass.
            # The dialog block (gather + tiny attention) is emitted interleaved
            # with seq 2's FFN chunks, and a CLS fixup pass (all 32 updated
            # CLS vectors, N=32 moving) rides along seq 3's FFN loops.
          with (
                tc.tile_pool(name="dlgw", bufs=1) as dlgw,
                tc.tile_pool(name="dlgp", bufs=1) as dlgp,
                tc.tile_pool(name="ffp", bufs=1) as ffp,
                tc.tile_pool(name="psZ", bufs=2, space="PSUM") as psZ,
                tc.tile_pool(name="psO", bufs=2, space="PSUM") as psO,
                tc.tile_pool(name="psL2", bufs=1, space="PSUM") as psL2,
                tc.tile_pool(name="psD", bufs=1, space="PSUM") as psD,
          ):
                nc.gpsimd.collective_compute(
                    "AllGather",
                    ALU.bypass,
                    replica_groups=[list(range(NCORES))],
                    ins=[cls_in.opt()],
                    outs=[cls_out.opt()],
                )
                dwq_s = dlgw.tile([128, HC, HID], BF, tag="dw", bufs=2)
                nc.sync.dma_start(dwq_s[:], d["dwq"][:])
                dwk_s = dlgw.tile([128, HC, HID], BF, tag="dw", bufs=2)
                nc.sync.dma_start(dwk_s[:], d["dwk"][:])
                dbv_rep = dlgw.tile([128, HID], BF, name="sb_dbv_rep")
                nc.sync.dma_start(dbv_rep[:], d["dbv_rep"][:])
                cmaskt_s = dlgw.tile([B, NH, B], F32, name="sb_cmaskt")
                nc.sync.dma_start(cmaskt_s[:], d["cmaskt"][:])

                def ffn_seq(s, fix):
                    x2 = fix  # x2clsT (bf16, WSC-scaled) when fix is set
                    # Wi runs in bf16 (same cycles as a split-fp8 pair, but
                    # no x/W quantization error); psum = WSC*(x@Wi) since x1
                    # carries WSC -> fold 1/WSC into the gelu input scale.
                    # Wo2 runs fp8 DoubleRow with hi+lo split weights; only
                    # the fp8 interT quantization error remains.
                    interT = ffp.tile([128, IC, S], F8, tag="interT", bufs=2)
                    if fix:
                        interC = ffp.tile([128, IC, B], F8, tag="interC")
                    for ic in range(IC):
                        pz = psZ.tile([128, S], F32, tag="pz")
                        if fix:
                            pzc = psZ.tile([128, B], F32, tag="pz")
                        for g in range(2):
                            nc.tensor.matmul(
                                pz[:], wi8_s[:, ic, 2 * g : 2 * g + 2, :],
                                x1q[:, s, 2 * g : 2 * g + 2, :],
                                start=(g == 0), stop=False, perf_mode=DR,
                            )
                            if fix:
                                nc.tensor.matmul(
                                    pzc[:], wi8_s[:, ic, 2 * g : 2 * g + 2, :],
                                    x2q[:, 2 * g : 2 * g + 2, :],
                                    start=(g == 0), stop=False, perf_mode=DR,
                                )
                        for hb in range(2):
                            nc.tensor.matmul(
                                pz[:], wib_s[:, ic, hb, :], x1[:, s, 4 + hb, :],
                                start=False, stop=(hb == 1),
                            )
                            if fix:
                                nc.tensor.matmul(
                                    pzc[:], wib_s[:, ic, hb, :], x2[:, 4 + hb, :],
                                    start=False, stop=(hb == 1),
                                )
                        nc.scalar.activation(
                            interT[:, ic, :], pz[:], AF.Gelu,
                            bias=bi_s[:, ic : ic + 1], scale=1.0 / (WSC * WSC),
                        )
                        if fix:
                            nc.scalar.activation(
                                interC[:, ic, :], pzc[:], AF.Gelu,
                                bias=bi_s[:, ic : ic + 1],
                                scale=1.0 / (WSC * WSC),
                            )
                        if ic % 2 == 1:
                            yield
                    y2 = ffp.tile([128, HC, S], F32R, tag="y2", bufs=2)
                    lnp2_m = psL2.tile([1, S], F32, tag="lnm")
                    lnp2_q = psL2.tile([1, S], F32, tag="lnq")
                    if fix:
                        y2c = ffp.tile([128, HC, B], F32R, tag="y2c")
                        lnc_m = psD.tile([1, B], F32, tag="pd")
                        lnc_q = psD.tile([1, B], F32, tag="pdo")
                    GI = IC // 2
                    for oc in range(HC):
                        wo2_sl = ffp.tile([128, IC, 128], F8, tag="wo2", bufs=2)
                        for g in range(3):
                            nc.sync.dma_start(
                                wo2_sl[:, 8 * g : 8 * g + 8, :],
                                d["wo2_hi"][:, oc, 8 * g : 8 * g + 8],
                            )
                        po = psO.tile([128, S], F32, tag="po")
                        if fix:
                            poc = psO.tile([128, B], F32, tag="po")
                        for g in range(GI):
                            nc.tensor.matmul(
                                po[:],
                                wo2_sl[:, 2 * g : 2 * g + 2, :],
                                interT[:, 2 * g : 2 * g + 2, :],
                                start=(g == 0),
                                stop=(g == GI - 1),
                                perf_mode=DR,
                            )
                            if fix:
                                nc.tensor.matmul(
                                    poc[:],
                                    wo2_sl[:, 2 * g : 2 * g + 2, :],
                                    interC[:, 2 * g : 2 * g + 2, :],
                                    start=(g == 0),
                                    stop=(g == GI - 1),
                                    perf_mode=DR,
                                )
                        nc.vector.scalar_tensor_tensor(
                            out=y2[:, oc, :], in0=po[:],
                            scalar=bo2_s[:, oc : oc + 1], in1=x1[:, s, oc, :],
                            op0=ALU.add, op1=ALU.add,
                        )
                        fsq = ffp.tile([128, S], F32R, tag="fsq", bufs=2)
                        nc.vector.tensor_tensor(
                            out=fsq[:], in0=y2[:, oc, :], in1=y2[:, oc, :],
                            op=ALU.mult,
                        )
                        nc.tensor.matmul(
                            lnp2_m[:], ones_fr[:, 0:1], y2[:, oc, :],
                            start=(oc == 0), stop=(oc == HC - 1),
                            skip_group_check=True,
                        )
                        nc.tensor.matmul(
                            lnp2_q[:], ones_fr[:, 0:1], fsq[:],
                            start=(oc == 0), stop=(oc == HC - 1),
                            skip_group_check=True,
                        )
                        if fix:
                            nc.vector.scalar_tensor_tensor(
                                out=y2c[:, oc, :], in0=poc[:],
                                scalar=bo2_s[:, oc : oc + 1], in1=x2[:, oc, :],
                                op0=ALU.add, op1=ALU.add,
                            )
                            fsqc = ffp.tile([128, B], F32R, tag="fsqc", bufs=2)
                            nc.vector.tensor_tensor(
                                out=fsqc[:], in0=y2c[:, oc, :], in1=y2c[:, oc, :],
                                op=ALU.mult,
                            )
                            nc.tensor.matmul(
                                lnc_m[:], ones_fr[:, 0:1], y2c[:, oc, :],
                                start=(oc == 0), stop=(oc == HC - 1),
                                skip_group_check=True,
                            )
                            nc.tensor.matmul(
                                lnc_q[:], ones_fr[:, 0:1], fsqc[:],
                                start=(oc == 0), stop=(oc == HC - 1),
                                skip_group_check=True,
                            )
                        yield
                    rstd_rep, msc_rep = _ln_reps(
                        nc, rows, lnp2_m, lnp2_q, eps_t, S, HID
                    )
                    outst = ffp.tile([128, HC, S], F32, tag="outst", bufs=2)
                    for oc in range(HC):
                        nc.vector.tensor_tensor(
                            out=outst[:, oc, :], in0=y2[:, oc, :],
                            in1=rstd_rep[:], op=ALU.mult,
                        )
                        nc.vector.tensor_tensor(
                            out=outst[:, oc, :], in0=outst[:, oc, :],
                            in1=msc_rep[:], op=ALU.subtract,
                        )
                        if oc % 2 == 1:
                            yield
                    for g in range(3):
                        nc.sync.dma_start(
                            d["out"][s][:, 2 * g : 2 * g + 2, 1:S],
                            outst[:, 2 * g : 2 * g + 2, 1:S],
                        )
                    if fix:
                        rsc, msc = _ln_reps(nc, rows, lnc_m, lnc_q, eps_t, B, HID)
                        outc = ffp.tile([128, HC, B], F32, tag="outc")
                        for oc in range(HC):
                            nc.vector.tensor_tensor(
                                out=outc[:, oc, :], in0=y2c[:, oc, :],
                                in1=rsc[:], op=ALU.mult,
                            )
                            nc.vector.tensor_tensor(
                                out=outc[:, oc, :], in0=outc[:, oc, :],
                                in1=msc[:], op=ALU.subtract,
                            )
                        outcl = ffp.tile([128, HC, 1, SPC], F32, tag="outcl")
                        pid = nc.partition_id()
                        nc.vector.tensor_copy(
                            out=outcl[:],
                            in_=outc.rearrange("p c (r s) -> p c r s", s=SPC)[
                                :, :, bass.ds(pid, 1), :
                            ],
                        )
                        for s2 in range(SPC):
                            nc.sync.dma_start(
                                d["out"][s2][:, :, 0:1], outcl[:, :, 0, s2 : s2 + 1]
                            )

                def dialog_stage():
                    # -------------------- dialog attention (tiny) ------------
                    clsF = dlgp.tile([128, HC, B], F32)
                    for r in range(NCORES):
                        nc.sync.dma_start(
                            clsF[:, :, r * SPC : (r + 1) * SPC],
                            cls_out[r * 128 : (r + 1) * 128, :, :],
                        )
                    clsT = dlgp.tile([128, HC, B], BF)
                    nc.vector.tensor_copy(out=clsT[:], in_=clsF[:])
                    yield

                    qdT = dlgp.tile([128, HC, B], BF)
                    kdT = dlgp.tile([128, HC, B], BF)
                    for dc in range(HC):
                        pq = psD.tile([128, B], F32, tag="pd")
                        for hc in range(HC):
                            nc.tensor.matmul(
                                pq[:], dwq_s[:, hc, dc * 128 : (dc + 1) * 128],
                                clsT[:, hc, :], start=(hc == 0), stop=(hc == HC - 1),
                            )
                        nc.vector.tensor_scalar_add(
                            out=qdT[:, dc, :], in0=pq[:], scalar1=dbq_s[:, dc : dc + 1]
                        )
                        yield
                        pk = psD.tile([128, B], F32, tag="pdo")
                        for hc in range(HC):
                            nc.tensor.matmul(
                                pk[:], dwk_s[:, hc, dc * 128 : (dc + 1) * 128],
                                clsT[:, hc, :], start=(hc == 0), stop=(hc == HC - 1),
                            )
                        nc.vector.tensor_scalar_add(
                            out=kdT[:, dc, :], in0=pk[:], scalar1=dbk_s[:, dc : dc + 1]
                        )
                        yield
                    dwv_s = dlgw.tile([128, HC, HID], BF, tag="dw", bufs=2)
                    nc.sync.dma_start(dwv_s[:], d["dwv"][:])
                    dwo_s = dlgw.tile([128, HC, HID], BF, tag="dw", bufs=2)
                    nc.sync.dma_start(dwo_s[:], d["dwo"][:])
                    # v natural [32, 768] + ones column per head
                    vd_aug = dlgp.tile([B, NH, HD + 1], BF)
                    nc.vector.memset(vd_aug[:, :, HD : HD + 1], 1.0)
                    for half in range(2):
                        pv = psD.tile([B, 384], F32, tag="pd")
                        for hc in range(HC):
                            nc.tensor.matmul(
                                pv[:], clsT[:, hc, :],
                                dwv_s[:, hc, half * 384 : (half + 1) * 384],
                                start=(hc == 0), stop=(hc == HC - 1),
                            )
                        nc.vector.tensor_tensor(
                            out=vd_aug[:, half * 6 : half * 6 + 6, 0:HD],
                            in0=pv[:].rearrange("p (h e) -> p h e", e=HD),
                            in1=dbv_rep[:B, half * 384 : (half + 1) * 384].rearrange(
                                "p (h e) -> p h e", e=HD
                            ),
                            op=ALU.add,
                        )
                        yield

                    # transposed scores: per-head matmuls (ping-pong between the
                    # two dialog psum banks), mask+scale on DVE, one batched exp.
                    sdt = dlgp.tile([B, NH, B], F32)
                    for h in range(NH):
                        dc, off = h // 2, (h % 2) * 64
                        pss = psD.tile([B, B], F32, tag=("pdo" if h % 2 == 0 else "pd"))
                        nc.tensor.matmul(
                            pss[:], kdT[off : off + 64, dc, :],
                            qdT[off : off + 64, dc, :], start=True, stop=True,
                        )
                        nc.vector.scalar_tensor_tensor(
                            out=sdt[:, h, :], in0=pss[:],
                            scalar=ISCALE / (WSC * WSC),
                            in1=cmaskt_s[:, h, :], op0=ALU.mult, op1=ALU.add,
                        )
                        if h % 3 == 2:
                            yield
                    probsTd = dlgp.tile([B, NH, B], BF)
                    nc.scalar.activation(probsTd[:], sdt[:], AF.Exp)
                    yield
                    ctxdT = dlgp.tile([128, HC, B], BF)
                    for h in range(NH):
                        dc, off = h // 2, (h % 2) * 64
                        pcd = psD.tile(
                            [HD + 1, B], F32, tag=("pdo" if h % 2 == 0 else "pd")
                        )
                        nc.tensor.matmul(
                            pcd[:], vd_aug[:, h, :], probsTd[:, h, :],
                            start=True, stop=True,
                        )
                        dnd = rows.tile([1, B], F32, tag="dnd", bufs=2)
                        nc.vector.tensor_copy(out=dnd[:], in_=pcd[HD : HD + 1, :])
                        rcpd = rows.tile([1, B], F32, tag="rcpd", bufs=2)
                        nc.vector.reciprocal_approx_fast(out=rcpd[:], in_=dnd[:])
                        repd = rows.tile([HD, B], F32, tag="repd", bufs=2)
                        nc.gpsimd.partition_broadcast(repd[:], rcpd[:])
                        nc.vector.tensor_tensor(
                            out=ctxdT[off : off + 64, dc, :],
                            in0=pcd[0:HD, :], in1=repd[:], op=ALU.mult,
                        )
                        if h % 3 == 2:
                            yield

                    # dialog output projection + residual + LN
                    # (po shares the psO banks with the FFN po rotation;
                    #  the LN stat accumulators take the psD ping-pong banks,
                    #  which have no other users from here to the fixup.)
                    ydT = dlgp.tile([128, HC, B], F32R)
                    lnpd_m = psD.tile([1, B], F32, tag="pd")
                    lnpd_q = psD.tile([1, B], F32, tag="pdo")
                    for oc in range(HC):
                        po = psO.tile([128, B], F32, tag="po")
                        for hc in range(HC):
                            nc.tensor.matmul(
                                po[:], dwo_s[:, hc, oc * 128 : (oc + 1) * 128],
                                ctxdT[:, hc, :], start=(hc == 0), stop=(hc == HC - 1),
                            )
                        nc.vector.scalar_tensor_tensor(
                            out=ydT[:, oc, :], in0=po[:], scalar=dbo_s[:, oc : oc + 1],
                            in1=clsF[:, oc, :], op0=ALU.add, op1=ALU.add,
                        )
                        dsq = dlgp.tile([128, B], F32R, tag="dsq", bufs=2)
                        nc.vector.tensor_tensor(
                            out=dsq[:], in0=ydT[:, oc, :], in1=ydT[:, oc, :],
                            op=ALU.mult,
                        )
                        nc.tensor.matmul(
                            lnpd_m[:], ones_fr[:, 0:1], ydT[:, oc, :],
                            start=(oc == 0), stop=(oc == HC - 1),
                            skip_group_check=True,
                        )
                        nc.tensor.matmul(
                            lnpd_q[:], ones_fr[:, 0:1], dsq[:],
                            start=(oc == 0), stop=(oc == HC - 1),
                            skip_group_check=True,
                        )
                        yield
                    rstd_rep, msc_rep = _ln_reps(
                        nc, rows, lnpd_m, lnpd_q, eps_t, B, HID, oscale=WSC
                    )
                    for oc in range(HC):
                        nc.vector.tensor_tensor(
                            out=x2clsT[:, oc, :], in0=ydT[:, oc, :], in1=rstd_rep[:],
                            op=ALU.mult,
                        )
                        nc.vector.tensor_tensor(
                            out=x2clsT[:, oc, :], in0=x2clsT[:, oc, :], in1=msc_rep[:],
                            op=ALU.subtract,
                        )
                        if oc < 4:
                            nc.vector.tensor_copy(
                                out=x2q[:, oc, :], in_=x2clsT[:, oc, :]
                            )
                        if oc % 2 == 1:
                            yield

                x2clsT = dlgp.tile([128, HC, B], BF)
                x2q = dlgp.tile([128, 4, B], F8)

                _drive(ffn_seq(0, None))
                _drive(ffn_seq(1, None))
                _drive(ffn_seq(2, None), dialog_stage())
                # last seq's FFN with the CLS fixup riding along
                _drive(ffn_seq(SPC - 1, x2clsT))


def _build():
    nc = bacc.Bacc(
        "TRN2", target_bir_lowering=False, debug=False, num_devices=NCORES
    )
    d = {}
    d["xbf"] = nc.dram_tensor("xbf", [SPC, 128, HC, S], BF, kind="ExternalInput")[:]
    d["x8"] = nc.dram_tensor("x8", [SPC, 128, HC, S], F8, kind="ExternalInput")[:]
    for nm in ["dwq", "dwk", "dwv", "dwo"]:
        d[nm] = nc.dram_tensor(nm, [128, HC, HID], BF, kind="ExternalInput")[:]
    for nm in ["wq_hi", "wk_hi", "wv_hi", "wao_hi"]:
        d[nm] = nc.dram_tensor(nm, [128, HC, HID], F8, kind="ExternalInput")[:]
    for nm in ["bq", "bk", "bao", "dbq", "dbk", "dbo", "bo2"]:
        d[nm] = nc.dram_tensor(nm, [128, HC], F32, kind="ExternalInput")[:]
    d["bv_rep"] = nc.dram_tensor("bv_rep", [128, HID], BF, kind="ExternalInput")[:]
    d["dbv_rep"] = nc.dram_tensor("dbv_rep", [128, HID], BF, kind="ExternalInput")[:]
    d["bi"] = nc.dram_tensor("bi", [128, IC], F32, kind="ExternalInput")[:]
    d["wi8"] = nc.dram_tensor("wi8", [128, IC, 4, 128], F8, kind="ExternalInput")[:]
    d["wib"] = nc.dram_tensor("wib", [128, IC, 2, 128], BF, kind="ExternalInput")[:]
    d["wo2_hi"] = nc.dram_tensor(
        "wo2_hi", [128, HC, IC, 128], F8, kind="ExternalInput"
    )[:]
    d["cmaskt"] = nc.dram_tensor("cmaskt", [B, NH, B], F32, kind="ExternalInput")[:]
    d["out"] = nc.dram_tensor("out", [SPC, 128, HC, S], F32, kind="ExternalOutput")[:]

    with tile.TileContext(nc, num_cores=NCORES) as tc:
        _emit(tc, d)
    nc.compile()
    return nc


def _np_bf16():
    import ml_dtypes

    return ml_dtypes.bfloat16


def _np_fp8():
    import ml_dtypes

    return ml_dtypes.float8_e4m3


def _pack_w(w):
    BF_NP = _np_bf16()
    return np.ascontiguousarray(
        np.asarray(w, np.float32).reshape(HC, 128, HID).transpose(1, 0, 2)
    ).astype(BF_NP)


def _pack_b(b, nch=HC):
    return np.ascontiguousarray(np.asarray(b, np.float32).reshape(nch, 128).T)


def _make_cmaskt():
    # additive mask, transposed [key, query], replicated per head.
    # -30 stands in for the reference's -10000 (exp(-30) ~ 9e-14 is
    # negligible next to any unmasked term, and row 0 - where every
    # in-dialog entry is masked - still reduces to softmax(s) exactly);
    # cross-dialog pairs use -60 so they stay negligible even against
    # fully-masked rows.
    pos = np.arange(TURNS)
    base = (pos[None, :] >= pos[:, None]).astype(np.float32) * (-30.0)
    cm = np.full((B, B), -60.0, np.float32)
    for dd in range(NDLG):
        cm[dd * TURNS : (dd + 1) * TURNS, dd * TURNS : (dd + 1) * TURNS] = base
    cmt = cm.T  # [key, query]
    return np.ascontiguousarray(np.tile(cmt[:, None, :], (1, NH, 1)))


_NC = None


def _get_nc():
    global _NC
    if _NC is None:
        _NC = _build()
    return _NC


def _pack_w_hilo(w):
    """fp8 hi/lo split of WSC*w in the [128, HC, HID] stationary layout."""
    FP8 = _np_fp8()
    wf = np.ascontiguousarray(
        WSC * np.asarray(w, np.float32).reshape(HC, 128, HID).transpose(1, 0, 2)
    )
    hi = wf.astype(FP8)
    lo = (wf - hi.astype(np.float32)).astype(FP8)
    return hi, lo


def _prepare_in_maps(inputs):
    BF_NP = _np_bf16()
    FP8_NP = _np_fp8()
    f = lambda k: np.asarray(inputs[k], np.float32)
    shared = {
        "dwq": _pack_w(f("dWq")),
        "dwk": _pack_w(f("dWk")),
        "dwv": _pack_w(f("dWv")),
        "dwo": _pack_w(f("dWo")),
        "bq": _pack_b(WSC * f("bq")),
        "bk": _pack_b(WSC * f("bk")),
        "bao": _pack_b(WSC * WSC * f("bao")),
        "dbq": _pack_b(WSC * f("dbq")),
        "dbk": _pack_b(WSC * f("dbk")),
        "dbo": _pack_b(WSC * f("dbo")),
        "bo2": _pack_b(WSC * f("bo2")),
        "bv_rep": np.ascontiguousarray(
            np.tile(WSC * f("bv").reshape(1, HID), (128, 1))
        ).astype(BF_NP),
        "dbv_rep": np.ascontiguousarray(
            np.tile(WSC * f("dbv").reshape(1, HID), (128, 1))
        ).astype(BF_NP),
        "bi": _pack_b(f("bi"), IC),
        "cmaskt": _make_cmaskt(),
    }
    wif = WSC * f("Wi").reshape(HC, 128, IC, 128).transpose(1, 2, 0, 3)
    shared["wi8"] = np.ascontiguousarray(wif[:, :, 0:4]).astype(FP8_NP)
    shared["wib"] = np.ascontiguousarray(wif[:, :, 4:6]).astype(BF_NP)
    for nm, key in [("wq", "Wq"), ("wk", "Wk"), ("wv", "Wv"), ("wao", "Wao")]:
        shared[nm + "_hi"] = _pack_w_hilo(f(key))[0]
    wo2f = np.ascontiguousarray(
        WSC * f("Wo2").reshape(IC, 128, HC, 128).transpose(1, 2, 0, 3)
    )
    shared["wo2_hi"] = wo2f.astype(FP8_NP)
    x = np.asarray(inputs["hidden_states"], np.float32)
    in_maps = []
    for c in range(NCORES):
        xs = x[c * SPC : (c + 1) * SPC]  # [4, 512, 768]
        xp = np.ascontiguousarray(
            xs.transpose(0, 2, 1).reshape(SPC, HC, 128, S).transpose(0, 2, 1, 3)
        )
        in_maps.append(
            {
                **shared,
                # xbf carries WSC^2 (the AO residual add matches the
                # WSC^2-scaled attention-output psum); x8 is the unscaled
                # fp8 GEMM operand.
                "xbf": (WSC * WSC * xp).astype(BF_NP),
                "x8": xp.astype(FP8_NP),
            }
        )
    return in_maps


def _assemble(results):
    parts = []
    for c in range(NCORES):
        o = np.asarray(results[c]["out"], np.float32)  # [4, 128, 6, 512]
        parts.append(o.transpose(0, 2, 1, 3).reshape(SPC, HID, S).transpose(0, 2, 1))
    return np.ascontiguousarray(np.concatenate(parts, axis=0))


def run(inputs, trace=False):
    nc = _get_nc()
    in_maps = _prepare_in_maps(inputs)
    res = run_bass_kernel_spmd(
        nc, in_maps, core_ids=list(range(NCORES)), trace=trace
    )
    return _assemble(res.results), res


def kernel(**inputs):
    out, _ = run(inputs)
    return out


# revision 88
# speedup vs baseline: 1.0442x; 1.0321x over previous
"""Trainium2 Bass kernel for nn_BertLayer_47339129536519.

BertLayer with hierarchical dialog attention:
  1) token-level MHA + SelfOutput(LN)       [B=32, S=512, H=768, 12 heads]
  2) dialog attention over per-turn CLS tokens (4 dialogs x 8 turns)
  3) FFN (gelu-erf) + output LN

Sharding: data-parallel over the 32 sequences, 4 per core on 8 cores.
The dialog attention mixes CLS vectors across cores -> tiny AllGather
(32x768) and every core redundantly computes the (tiny) dialog block.

v9 vs v2 (the 885us baseline):
  * the chip runs power-throttled (avg tensor-util limit ~70%), so
    wall time ~ PE-busy / 0.7: every tensor-engine cycle cut pays 1.4x.
  * fp8e4 DoubleRow matmuls (2 k-tiles per pass ~ 2x bf16 FLOPs) for
    the V/Q/K/AO projections (plain fp8: softmax+LN+residual damp the
    quantization to ~1e-3 of the output), for Wo2 (fp8 interT), and
    for 4 of Wi's 6 k-chunks (hybrid: error scales sqrt(2/3)).
    Weights pre-scaled x32 into e4m3's normal range; the scale rides
    psum and is folded into gelu/exp input scales and LN scale
    invariance (x carries x1024 via the host, x1 carries x32).
  * phase 1 software-pipelined: projections of seq s+1 interleave with
    attention (scores/exp/PV/AO/LN) of seq s via weighted round-robin
    generator emission, keeping the PE fed while ACT chews exp.
  * LayerNorm rstd: ACT Sqrt + reciprocal_approx_fast + one Newton
    step (the raw approx's ~4e-3 rstd error scales the output 1:1).
  * dialog attention emitted interleaved with FFN(seq2) chunks so its
    skinny dependency chain doesn't head-of-line-block the PE queue;
    CLS fixup rides FFN(seq3) (stale-CLS trick for columns 1..S-1).
  * qt bias + softmax denominator copies on DVE (ACT is exp-bound).
"""

import numpy as np

import concourse.bass as bass
import concourse.mybir as mybir
import concourse.tile as tile
from concourse import bacc
from concourse.bass_utils import run_bass_kernel_spmd

HID, NH, HD, S = 768, 12, 64, 512
B, NCORES, SPC = 32, 8, 4  # batch, cores, sequences per core
TURNS = 8
NDLG = B // TURNS  # 4 dialogs
HC = HID // 128  # 6 hidden-dim chunks of 128
IC = (4 * HID) // 128  # 24 intermediate chunks
INTER = 4 * HID  # 3072
EPS = 1e-12
ISCALE = 0.125  # 1/sqrt(64)

F32 = mybir.dt.float32
F32R = mybir.dt.float32r
BF = mybir.dt.bfloat16
F8 = mybir.dt.float8e4
DR = mybir.MatmulPerfMode.DoubleRow
AF = mybir.ActivationFunctionType
ALU = mybir.AluOpType
AX = mybir.AxisListType
WSC = 32.0  # fp8 weight pre-scale (0.02-sigma weights -> normal e4m3 range)


def _drive(*gens, weights=None):
    """Weighted round-robin drive: interleaves generator emission so
    independent work lands between dependent chains in each engine's
    (in-order) queue.  weights[i] = how many steps of gens[i] per cycle
    (fractional allowed: 0.5 = one step every other cycle)."""
    live = [(g, (weights[i] if weights else 1.0))
            for i, g in enumerate(gens) if g is not None]
    credit = [0.0] * len(live)
    while live:
        for i, (g, w) in enumerate(list(live)):
            if g is None:
                continue
            credit[i] += w
            while credit[i] >= 1.0 and g is not None:
                credit[i] -= 1.0
                try:
                    next(g)
                except StopIteration:
                    live[i] = (None, w)
                    g = None
        if all(g is None for g, _ in live):
            break


def _ln_reps(nc, rows, lnp_m, lnp_q, eps_t, n, dim, oscale=1.0):
    """From accumulated sum (lnp_m[1,n]) / sum-of-squares (lnp_q[1,n]) psum
    rows, produce broadcast [128, n] tiles (rstd_rep, mscaled_rep) so that
    oscale*normalized = y * rstd_rep - mscaled_rep.  The reciprocal is
    approx_fast + one Newton step (error ~(4e-3)^2, vs 4e-3 for the raw
    approx, which directly scales the LN output).  oscale folds into the
    Newton bracket for free.  LN is scale-invariant in y, so callers can
    feed pre-scaled y without adjusting anything here."""
    # scratch rows packed on partitions of one tile: [1,n] tiles cost a
    # full 2KB of per-partition address space each.  mean/rstd stay
    # partition-0 tiles (partition_broadcast reads partition 0).
    # (two SBUF inputs of a DVE op must share base partition -> keep all
    #  row tiles at partition 0; fold intermediates in place)
    mean = rows.tile([1, n], F32, tag="ln_mean", bufs=1)
    nc.vector.tensor_scalar_mul(mean[:], lnp_m[:], 1.0 / dim)
    rstd = rows.tile([1, n], F32, tag="ln_rstd", bufs=1)
    # rstd holds mean^2 transiently
    nc.vector.tensor_tensor(out=rstd[:], in0=mean[:], in1=mean[:], op=ALU.mult)
    var = rows.tile([1, n], F32, tag="ln_var", bufs=1)
    nc.vector.scalar_tensor_tensor(
        out=var[:], in0=lnp_q[:], scalar=1.0 / dim, in1=rstd[:],
        op0=ALU.mult, op1=ALU.subtract,
    )
    nc.scalar.activation(var[:], var[:], AF.Sqrt, bias=eps_t[:])
    r0 = rows.tile([1, n], F32, tag="ln_r0", bufs=1)
    nc.vector.reciprocal_approx_fast(out=r0[:], in_=var[:])
    # Newton: rstd = r0 * (2 - var * r0); var becomes the bracket in place
    nc.vector.tensor_tensor(out=var[:], in0=var[:], in1=r0[:], op=ALU.mult)
    nc.vector.tensor_scalar(
        out=var[:], in0=var[:], scalar1=-oscale, scalar2=2.0 * oscale,
        op0=ALU.mult, op1=ALU.add,
    )
    nc.vector.tensor_tensor(out=rstd[:], in0=r0[:], in1=var[:], op=ALU.mult)
    nc.vector.tensor_tensor(out=mean[:], in0=mean[:], in1=rstd[:], op=ALU.mult)
    rstd_rep = rows.tile([128, n], F32, tag="ln_rstd_rep", bufs=1)
    nc.gpsimd.partition_broadcast(rstd_rep[:], rstd[:])
    msc_rep = rows.tile([128, n], F32, tag="ln_msc_rep", bufs=1)
    nc.gpsimd.partition_broadcast(msc_rep[:], mean[:])
    return rstd_rep, msc_rep


def _emit(tc, d):
    nc = tc.nc
    from concourse import library_config

    nc.gpsimd.load_library(library_config.attn)  # for partition_broadcast

    with (
        tc.tile_pool(name="setup", bufs=1) as setup,
        tc.tile_pool(name="rows", bufs=2) as rows,
        tc.tile_pool(name="dram", bufs=1, space="DRAM") as dram,
    ):
        # ---- constants / small params ----
        ones_f32 = setup.tile([128, 2], F32)
        nc.vector.memset(ones_f32, 1.0)
        ones_fr = ones_f32.bitcast(F32R)
        ones_bf = setup.tile([128, 2], BF)
        nc.vector.memset(ones_bf, 1.0)
        eps_t = setup.tile([1, 1], F32)
        nc.vector.memset(eps_t, EPS)

        def load_small(name, dt=F32):
            t = setup.tile(list(d[name].shape), dt, name="sb_" + name)
            nc.sync.dma_start(t[:], d[name][:])
            return t

        bq_s = load_small("bq")
        bk_s = load_small("bk")
        bao_s = load_small("bao")
        bv_rep = load_small("bv_rep", BF)
        dbq_s = load_small("dbq")
        dbk_s = load_small("dbk")
        dbo_s = load_small("dbo")
        bi_s = load_small("bi")
        bo2_s = load_small("bo2")

        # persistent-through-kernel tiles.  x1 holds 32*LN1out (bf16 is
        # scale-free; LN2 washes the factor out).  x1q: fp8 copy of hid
        # chunks 0-3 for the hybrid-precision Wi GEMM (4 chunks fp8
        # DoubleRow + 2 chunks bf16 -> 2/3 of the x/W quantization noise
        # at 2/3 of the bf16 cycles).
        x1 = setup.tile([128, SPC, HC, S], BF)
        x1q = setup.tile([128, SPC, 4, S], F8)
        cls_in = dram.tile([128, HC, SPC], F32, name="cls_in")
        cls_out = dram.tile([NCORES * 128, HC, SPC], F32, name="cls_out")

        # FFN Wi weights: resident, DMA overlapped with phase 1
        with tc.tile_pool(name="ffw", bufs=1) as ffw:
          wi8_s = ffw.tile([128, IC, 4, 128], F8)
          wib_s = ffw.tile([128, IC, 2, 128], BF)
          # ========================= PHASE 1: token attention =================
          with (
            tc.tile_pool(name="attw", bufs=1) as attw,
            tc.tile_pool(name="attp", bufs=1) as attp,
            tc.tile_pool(name="psP", bufs=2, space="PSUM") as psP,
            tc.tile_pool(name="psS", bufs=4, space="PSUM") as psS,
            tc.tile_pool(name="psC", bufs=2, space="PSUM") as psC,
          ):
            # attention weights in plain fp8 (WSC-scaled): softmax + LN +
            # residual damp the quantization to ~1e-3 of the output, and
            # the DoubleRow matmuls run the projections at 2x bf16 rate.
            attws = {}
            for nm in ["wv", "wq", "wk", "wao"]:
                t = attw.tile([128, HC, HID], F8, name="sb_" + nm)
                nc.sync.dma_start(t[:], d[nm + "_hi"][:])
                attws[nm] = t
            for g in range(8):
                nc.sync.dma_start(
                    wi8_s[:, g * 3 : (g + 1) * 3, :, :],
                    d["wi8"][:, g * 3 : (g + 1) * 3],
                )
                nc.sync.dma_start(
                    wib_s[:, g * 3 : (g + 1) * 3, :, :],
                    d["wib"][:, g * 3 : (g + 1) * 3],
                )

            xtiles = {}

            def load_x(si):
                if si >= SPC:
                    return
                t = attp.tile([128, HC, S], BF, tag="xs", bufs=2, name="xs")
                for g in range(3):
                    nc.sync.dma_start(
                        t[:, 2 * g : 2 * g + 2, :], d["xbf"][si, :, 2 * g : 2 * g + 2]
                    )
                t8 = attp.tile([128, HC, S], F8, tag="xs8", bufs=2, name="xs8")
                nc.sync.dma_start(t8[:], d["x8"][si])
                xtiles[si] = (t, t8)

            # per-seq projection outputs (2 generations live: s and s+1)
            vtiles, qtiles, ktiles = {}, {}, {}

            def proj_stage(s):
                """V/Q/K projections for seq s: fp8 DoubleRow over hc pairs,
                hi then lo weight halves accumulating into one psum chain.
                psum comes out at WSC*(x@W); the bias evictions rescale."""
                xs, xs8 = xtiles[s]
                v_aug = attp.tile([128, 4, NH, HD + 1], BF, tag="vaug", bufs=2)
                nc.vector.memset(v_aug[:, :, :, HD : HD + 1], 1.0)
                vtiles[s] = v_aug
                GH = HC // 2
                wv8 = attws["wv"]
                for sc in range(4):
                    for half in range(2):
                        pv = psP.tile([128, S], F32, tag="psP")
                        for g in range(GH):
                            nc.tensor.matmul(
                                pv[:, :384],
                                xs8[:, 2 * g : 2 * g + 2,
                                    sc * 128 : (sc + 1) * 128],
                                wv8[:, 2 * g : 2 * g + 2,
                                    half * 384 : (half + 1) * 384],
                                start=(g == 0),
                                stop=(g == GH - 1),
                                perf_mode=DR,
                            )
                        nc.vector.tensor_tensor(
                            out=v_aug[:, sc, half * 6 : half * 6 + 6, 0:HD],
                            in0=pv[:, :384].rearrange("p (h e) -> p h e", e=HD),
                            in1=bv_rep[
                                :, half * 384 : (half + 1) * 384
                            ].rearrange("p (h e) -> p h e", e=HD),
                            op=ALU.add,
                        )
                        yield
                qt = attp.tile([128, HC, S], F8, tag="qt", bufs=2)
                kt = attp.tile([128, HC, S], F8, tag="kt", bufs=2)
                qtiles[s], ktiles[s] = qt, kt
                wq8, wk8 = attws["wq"], attws["wk"]
                for dc in range(HC):
                    pq = psP.tile([128, S], F32, tag="psP")
                    for g in range(GH):
                        nc.tensor.matmul(
                            pq[:],
                            wq8[:, 2 * g : 2 * g + 2, dc * 128 : (dc + 1) * 128],
                            xs8[:, 2 * g : 2 * g + 2, :],
                            start=(g == 0),
                            stop=(g == GH - 1),
                            perf_mode=DR,
                        )
                    nc.vector.tensor_scalar_add(
                        out=qt[:, dc, :], in0=pq[:], scalar1=bq_s[:, dc : dc + 1]
                    )
                    yield
                    pk = psP.tile([128, S], F32, tag="psP")
                    for g in range(GH):
                        nc.tensor.matmul(
                            pk[:],
                            wk8[:, 2 * g : 2 * g + 2, dc * 128 : (dc + 1) * 128],
                            xs8[:, 2 * g : 2 * g + 2, :],
                            start=(g == 0),
                            stop=(g == GH - 1),
                            perf_mode=DR,
                        )
                    nc.vector.tensor_scalar_add(
                        out=kt[:, dc, :], in0=pk[:], scalar1=bk_s[:, dc : dc + 1]
                    )
                    yield

            def attn_stage(s):
                """scores/softmax/PV/AO/LN1 for seq s (needs proj_stage(s)
                complete). ACT(exp)-dominated; meant to overlap
                proj_stage(s+1) on the PE."""
                xs, _xs8 = xtiles.pop(s)
                v_aug = vtiles.pop(s)
                qt = qtiles.pop(s)
                kt = ktiles.pop(s)
                load_x(s + 2)

                # ctxT carries WSC*ctx (v_aug holds WSC*v with a 1.0 ones
                # column, so the PV ratio comes out WSC-scaled), fp8 for
                # the DoubleRow AO projection.
                ctxT = attp.tile([128, HC, S], F8, tag="ctxT", bufs=1)

                def emit_scores(dc):
                    probsT = attp.tile(
                        [128, 4, 2, S], BF, tag="probsT", bufs=2, name="probsT"
                    )
                    for kc in range(4):
                        pse = psS.tile([128, S], F32, tag="ps_s")
                        pso = psS.tile([128, S], F32, tag="ps_s")
                        nc.tensor.matmul(
                            pse[:],
                            kt[0:64, dc, kc * 128 : (kc + 1) * 128],
                            qt[0:64, dc, :],
                            start=True, stop=True,
                        )
                        nc.tensor.matmul(
                            pso[:],
                            kt[64:128, dc, kc * 128 : (kc + 1) * 128],
                            qt[64:128, dc, :],
                            start=True, stop=True,
                        )
                        nc.scalar.activation(
                            probsT[:, kc, 0, :], pse[:], AF.Exp,
                            scale=ISCALE / (WSC * WSC),
                        )
                        nc.scalar.activation(
                            probsT[:, kc, 1, :], pso[:], AF.Exp,
                            scale=ISCALE / (WSC * WSC),
                        )
                    return probsT

                def emit_pv(dc, probsT, eo):
                    h = 2 * dc + eo
                    pc = psC.tile([HD + 1, S], F32, tag="pc")
                    for kc in range(4):
                        nc.tensor.matmul(
                            pc[:],
                            v_aug[:, kc, h, :],
                            probsT[:, kc, eo, :],
                            start=(kc == 0),
                            stop=(kc == 3),
                        )
                    dn = rows.tile([1, S], F32, tag="dn", bufs=2)
                    nc.vector.tensor_copy(out=dn[:], in_=pc[HD : HD + 1, :])
                    rcp = rows.tile([1, S], F32, tag="rcp", bufs=2)
                    nc.vector.reciprocal_approx_fast(out=rcp[:], in_=dn[:])
                    rep = rows.tile([HD, S], F32, tag="rep")
                    nc.gpsimd.partition_broadcast(rep[:], rcp[:])
                    nc.vector.tensor_tensor(
                        out=ctxT[eo * 64 : eo * 64 + 64, dc, :],
                        in0=pc[0:HD, :],
                        in1=rep[:],
                        op=ALU.mult,
                    )

                prev = None
                for dc in range(HC):
                    pt = emit_scores(dc)
                    yield
                    if prev is not None:
                        emit_pv(prev[0], prev[1], 0)
                        yield
                        emit_pv(prev[0], prev[1], 1)
                        yield
                    prev = (dc, pt)
                emit_pv(prev[0], prev[1], 0)
                yield
                emit_pv(prev[0], prev[1], 1)
                yield

                # ---- attention output projection + residual + LN1 ----
                # LN stat rows share the psC "pc" rotation (the 12 PV tiles
                # precede them each seq; released at _ln_reps before the
                # next seq's PV allocations come around).
                y = attp.tile([128, HC, S], F32R, tag="y")
                lnp_m = psC.tile([1, S], F32, tag="pc")
                lnp_q = psC.tile([1, S], F32, tag="pc")
                GH = HC // 2
                wao8 = attws["wao"]
                for dc in range(HC):
                    pa = psP.tile([128, S], F32, tag="psP")
                    for g in range(GH):
                        nc.tensor.matmul(
                            pa[:],
                            wao8[:, 2 * g : 2 * g + 2, dc * 128 : (dc + 1) * 128],
                            ctxT[:, 2 * g : 2 * g + 2, :],
                            start=(g == 0),
                            stop=(g == GH - 1),
                            perf_mode=DR,
                        )
                    nc.vector.scalar_tensor_tensor(
                        out=y[:, dc, :], in0=pa[:], scalar=bao_s[:, dc : dc + 1],
                        in1=xs[:, dc, :], op0=ALU.add, op1=ALU.add,
                    )
                    sq = attp.tile([128, S], BF, tag="sq", bufs=1)
                    nc.vector.tensor_tensor(
                        out=sq[:], in0=y[:, dc, :], in1=y[:, dc, :], op=ALU.mult
                    )
                    nc.tensor.matmul(
                        lnp_m[:], ones_fr[:, 0:1], y[:, dc, :],
                        start=(dc == 0), stop=(dc == HC - 1),
                        skip_group_check=True,
                    )
                    nc.tensor.matmul(
                        lnp_q[:], ones_bf[:, 0:1], sq[:],
                        start=(dc == 0), stop=(dc == HC - 1),
                        skip_group_check=True,
                    )
                    yield
                rstd_rep, msc_rep = _ln_reps(
                    nc, rows, lnp_m, lnp_q, eps_t, S, HID, oscale=WSC
                )
                for dc in range(HC):
                    nc.vector.tensor_tensor(
                        out=x1[:, s, dc, :], in0=y[:, dc, :], in1=rstd_rep[:],
                        op=ALU.mult,
                    )
                    nc.vector.tensor_tensor(
                        out=x1[:, s, dc, :], in0=x1[:, s, dc, :], in1=msc_rep[:],
                        op=ALU.subtract,
                    )
                    if dc < 4:
                        nc.vector.tensor_copy(
                            out=x1q[:, s, dc, :], in_=x1[:, s, dc, :]
                        )
                    yield
                clsst = rows.tile([128, HC, 1], F32, tag="clsst")
                nc.vector.tensor_copy(out=clsst[:], in_=x1[:, s, :, 0:1])
                nc.sync.dma_start(cls_in[:, :, s : s + 1], clsst[:])

            load_x(0)
            load_x(1)
            _drive(proj_stage(0))
            for s in range(SPC):
                _drive(
                    attn_stage(s),
                    proj_stage(s + 1) if s + 1 < SPC else None,
                )

          # ========= PHASE 2+3: dialog attention overlapped with FFN =========
            #
            # FFN for all 4 seqs runs on x1 with the *stale* CLS column; every
            # token's FFN+LN2 is independent, so only column 0 of each output
            # is affected - and column 0 is not stored from the main pass.
            # The dialog block (gather + tiny attention) is emitted interleaved
            # with seq 2's FFN chunks, and a CLS fixup pass (all 32 updated
            # CLS vectors, N=32 moving) rides along seq 3's FFN loops.
          with (
                tc.tile_pool(name="dlgw", bufs=1) as dlgw,
                tc.tile_pool(name="dlgp", bufs=1) as dlgp,
                tc.tile_pool(name="ffp", bufs=1) as ffp,
                tc.tile_pool(name="psZ", bufs=2, space="PSUM") as psZ,
                tc.tile_pool(name="psO", bufs=2, space="PSUM") as psO,
                tc.tile_pool(name="psL2", bufs=1, space="PSUM") as psL2,
                tc.tile_pool(name="psD", bufs=1, space="PSUM") as psD,
          ):
                nc.gpsimd.collective_compute(
                    "AllGather",
                    ALU.bypass,
                    replica_groups=[list(range(NCORES))],
                    ins=[cls_in.opt()],
                    outs=[cls_out.opt()],
                )
                dwq_s = dlgw.tile([128, HC, HID], BF, tag="dw", bufs=2)
                nc.sync.dma_start(dwq_s[:], d["dwq"][:])
                dwk_s = dlgw.tile([128, HC, HID], BF, tag="dw", bufs=2)
                nc.sync.dma_start(dwk_s[:], d["dwk"][:])
                dbv_rep = dlgw.tile([128, HID], BF, name="sb_dbv_rep")
                nc.sync.dma_start(dbv_rep[:], d["dbv_rep"][:])
                cmaskt_s = dlgw.tile([B, NH, B], F32, name="sb_cmaskt")
                nc.sync.dma_start(cmaskt_s[:], d["cmaskt"][:])

                def ffn_seq(s, fix):
                    x2 = fix  # x2clsT (bf16, WSC-scaled) when fix is set
                    # Wi runs in bf16 (same cycles as a split-fp8 pair, but
                    # no x/W quantization error); psum = WSC*(x@Wi) since x1
                    # carries WSC -> fold 1/WSC into the gelu input scale.
                    # Wo2 runs fp8 DoubleRow with hi+lo split weights; only
                    # the fp8 interT quantization error remains.
                    interT = ffp.tile([128, IC, S], F8, tag="interT", bufs=2)
                    if fix:
                        interC = ffp.tile([128, IC, B], F8, tag="interC")
                    for ic in range(IC):
                        pz = psZ.tile([128, S], F32, tag="pz")
                        if fix:
                            pzc = psZ.tile([128, B], F32, tag="pz")
                        for g in range(2):
                            nc.tensor.matmul(
                                pz[:], wi8_s[:, ic, 2 * g : 2 * g + 2, :],
                                x1q[:, s, 2 * g : 2 * g + 2, :],
                                start=(g == 0), stop=False, perf_mode=DR,
                            )
                            if fix:
                                nc.tensor.matmul(
                                    pzc[:], wi8_s[:, ic, 2 * g : 2 * g + 2, :],
                                    x2q[:, 2 * g : 2 * g + 2, :],
                                    start=(g == 0), stop=False, perf_mode=DR,
                                )
                        for hb in range(2):
                            nc.tensor.matmul(
                                pz[:], wib_s[:, ic, hb, :], x1[:, s, 4 + hb, :],
                                start=False, stop=(hb == 1),
                            )
                            if fix:
                                nc.tensor.matmul(
                                    pzc[:], wib_s[:, ic, hb, :], x2[:, 4 + hb, :],
                                    start=False, stop=(hb == 1),
                                )
                        nc.scalar.activation(
                            interT[:, ic, :], pz[:], AF.Gelu,
                            bias=bi_s[:, ic : ic + 1], scale=1.0 / (WSC * WSC),
                        )
                        if fix:
                            nc.scalar.activation(
                                interC[:, ic, :], pzc[:], AF.Gelu,
                                bias=bi_s[:, ic : ic + 1],
                                scale=1.0 / (WSC * WSC),
                            )
                        if ic % 2 == 1:
                            yield
                    y2 = ffp.tile([128, HC, S], F32R, tag="y2", bufs=2)
                    lnp2_m = psL2.tile([1, S], F32, tag="lnm")
                    lnp2_q = psL2.tile([1, S], F32, tag="lnq")
                    if fix:
                        y2c = ffp.tile([128, HC, B], F32R, tag="y2c")
                        lnc_m = psD.tile([1, B], F32, tag="pd")
                        lnc_q = psD.tile([1, B], F32, tag="pdo")
                    GI = IC // 2
                    for oc in range(HC):
                        wo2_sl = ffp.tile([128, IC, 128], F8, tag="wo2", bufs=2)
                        for g in range(3):
                            nc.sync.dma_start(
                                wo2_sl[:, 8 * g : 8 * g + 8, :],
                                d["wo2_hi"][:, oc, 8 * g : 8 * g + 8],
                            )
                        po = psO.tile([128, S], F32, tag="po")
                        if fix:
                            poc = psO.tile([128, B], F32, tag="po")
                        for g in range(GI):
                            nc.tensor.matmul(
                                po[:],
                                wo2_sl[:, 2 * g : 2 * g + 2, :],
                                interT[:, 2 * g : 2 * g + 2, :],
                                start=(g == 0),
                                stop=(g == GI - 1),
                                perf_mode=DR,
                            )
                            if fix:
                                nc.tensor.matmul(
                                    poc[:],
                                    wo2_sl[:, 2 * g : 2 * g + 2, :],
                                    interC[:, 2 * g : 2 * g + 2, :],
                                    start=(g == 0),
                                    stop=(g == GI - 1),
                                    perf_mode=DR,
                                )
                        nc.vector.scalar_tensor_tensor(
                            out=y2[:, oc, :], in0=po[:],
                            scalar=bo2_s[:, oc : oc + 1], in1=x1[:, s, oc, :],
                            op0=ALU.add, op1=ALU.add,
                        )
                        fsq = ffp.tile([128, S], F32R, tag="fsq", bufs=2)
                        nc.vector.tensor_tensor(
                            out=fsq[:], in0=y2[:, oc, :], in1=y2[:, oc, :],
                            op=ALU.mult,
                        )
                        nc.tensor.matmul(
                            lnp2_m[:], ones_fr[:, 0:1], y2[:, oc, :],
                            start=(oc == 0), stop=(oc == HC - 1),
                            skip_group_check=True,
                        )
                        nc.tensor.matmul(
                            lnp2_q[:], ones_fr[:, 0:1], fsq[:],
                            start=(oc == 0), stop=(oc == HC - 1),
                            skip_group_check=True,
                        )
                        if fix:
                            nc.vector.scalar_tensor_tensor(
                                out=y2c[:, oc, :], in0=poc[:],
                                scalar=bo2_s[:, oc : oc + 1], in1=x2[:, oc, :],
                                op0=ALU.add, op1=ALU.add,
                            )
                            fsqc = ffp.tile([128, B], F32R, tag="fsqc", bufs=2)
                            nc.vector.tensor_tensor(
                                out=fsqc[:], in0=y2c[:, oc, :], in1=y2c[:, oc, :],
                                op=ALU.mult,
                            )
                            nc.tensor.matmul(
                                lnc_m[:], ones_fr[:, 0:1], y2c[:, oc, :],
                                start=(oc == 0), stop=(oc == HC - 1),
                                skip_group_check=True,
                            )
                            nc.tensor.matmul(
                                lnc_q[:], ones_fr[:, 0:1], fsqc[:],
                                start=(oc == 0), stop=(oc == HC - 1),
                                skip_group_check=True,
                            )
                        yield
                    rstd_rep, msc_rep = _ln_reps(
                        nc, rows, lnp2_m, lnp2_q, eps_t, S, HID
                    )
                    outst = ffp.tile([128, HC, S], F32, tag="outst", bufs=2)
                    for oc in range(HC):
                        nc.vector.tensor_tensor(
                            out=outst[:, oc, :], in0=y2[:, oc, :],
                            in1=rstd_rep[:], op=ALU.mult,
                        )
                        nc.vector.tensor_tensor(
                            out=outst[:, oc, :], in0=outst[:, oc, :],
                            in1=msc_rep[:], op=ALU.subtract,
                        )
                        if oc % 2 == 1:
                            yield
                    for g in range(3):
                        nc.sync.dma_start(
                            d["out"][s][:, 2 * g : 2 * g + 2, 1:S],
                            outst[:, 2 * g : 2 * g + 2, 1:S],
                        )
                    if fix:
                        rsc, msc = _ln_reps(nc, rows, lnc_m, lnc_q, eps_t, B, HID)
                        outc = ffp.tile([128, HC, B], F32, tag="outc")
                        for oc in range(HC):
                            nc.vector.tensor_tensor(
                                out=outc[:, oc, :], in0=y2c[:, oc, :],
                                in1=rsc[:], op=ALU.mult,
                            )
                            nc.vector.tensor_tensor(
                                out=outc[:, oc, :], in0=outc[:, oc, :],
                                in1=msc[:], op=ALU.subtract,
                            )
                        outcl = ffp.tile([128, HC, 1, SPC], F32, tag="outcl")
                        pid = nc.partition_id()
                        nc.vector.tensor_copy(
                            out=outcl[:],
                            in_=outc.rearrange("p c (r s) -> p c r s", s=SPC)[
                                :, :, bass.ds(pid, 1), :
                            ],
                        )
                        for s2 in range(SPC):
                            nc.sync.dma_start(
                                d["out"][s2][:, :, 0:1], outcl[:, :, 0, s2 : s2 + 1]
                            )

                def dialog_stage():
                    # -------------------- dialog attention (tiny) ------------
                    clsF = dlgp.tile([128, HC, B], F32)
                    for r in range(NCORES):
                        nc.sync.dma_start(
                            clsF[:, :, r * SPC : (r + 1) * SPC],
                            cls_out[r * 128 : (r + 1) * 128, :, :],
                        )
                    clsT = dlgp.tile([128, HC, B], BF)
                    nc.vector.tensor_copy(out=clsT[:], in_=clsF[:])
                    yield

                    qdT = dlgp.tile([128, HC, B], BF)
                    kdT = dlgp.tile([128, HC, B], BF)
                    for dc in range(HC):
                        pq = psD.tile([128, B], F32, tag="pd")
                        for hc in range(HC):
                            nc.tensor.matmul(
                                pq[:], dwq_s[:, hc, dc * 128 : (dc + 1) * 128],
                                clsT[:, hc, :], start=(hc == 0), stop=(hc == HC - 1),
                            )
                        nc.vector.tensor_scalar_add(
                            out=qdT[:, dc, :], in0=pq[:], scalar1=dbq_s[:, dc : dc + 1]
                        )
                        yield
                        pk = psD.tile([128, B], F32, tag="pdo")
                        for hc in range(HC):
                            nc.tensor.matmul(
                                pk[:], dwk_s[:, hc, dc * 128 : (dc + 1) * 128],
                                clsT[:, hc, :], start=(hc == 0), stop=(hc == HC - 1),
                            )
                        nc.vector.tensor_scalar_add(
                            out=kdT[:, dc, :], in0=pk[:], scalar1=dbk_s[:, dc : dc + 1]
                        )
                        yield
                    dwv_s = dlgw.tile([128, HC, HID], BF, tag="dw", bufs=2)
                    nc.sync.dma_start(dwv_s[:], d["dwv"][:])
                    dwo_s = dlgw.tile([128, HC, HID], BF, tag="dw", bufs=2)
                    nc.sync.dma_start(dwo_s[:], d["dwo"][:])
                    # v natural [32, 768] + ones column per head
                    vd_aug = dlgp.tile([B, NH, HD + 1], BF)
                    nc.vector.memset(vd_aug[:, :, HD : HD + 1], 1.0)
                    for half in range(2):
                        pv = psD.tile([B, 384], F32, tag="pd")
                        for hc in range(HC):
                            nc.tensor.matmul(
                                pv[:], clsT[:, hc, :],
                                dwv_s[:, hc, half * 384 : (half + 1) * 384],
                                start=(hc == 0), stop=(hc == HC - 1),
                            )
                        nc.vector.tensor_tensor(
                            out=vd_aug[:, half * 6 : half * 6 + 6, 0:HD],
                            in0=pv[:].rearrange("p (h e) -> p h e", e=HD),
                            in1=dbv_rep[:B, half * 384 : (half + 1) * 384].rearrange(
                                "p (h e) -> p h e", e=HD
                            ),
                            op=ALU.add,
                        )
                        yield

                    # transposed scores: per-head matmuls (ping-pong between the
                    # two dialog psum banks), mask+scale on DVE, one batched exp.
                    sdt = dlgp.tile([B, NH, B], F32)
                    for h in range(NH):
                        dc, off = h // 2, (h % 2) * 64
                        pss = psD.tile([B, B], F32, tag=("pdo" if h % 2 == 0 else "pd"))
                        nc.tensor.matmul(
                            pss[:], kdT[off : off + 64, dc, :],
                            qdT[off : off + 64, dc, :], start=True, stop=True,
                        )
                        nc.vector.scalar_tensor_tensor(
                            out=sdt[:, h, :], in0=pss[:],
                            scalar=ISCALE / (WSC * WSC),
                            in1=cmaskt_s[:, h, :], op0=ALU.mult, op1=ALU.add,
                        )
                        if h % 3 == 2:
                            yield
                    probsTd = dlgp.tile([B, NH, B], BF)
                    nc.scalar.activation(probsTd[:], sdt[:], AF.Exp)
                    yield
                    ctxdT = dlgp.tile([128, HC, B], BF)
                    for h in range(NH):
                        dc, off = h // 2, (h % 2) * 64
                        pcd = psD.tile(
                            [HD + 1, B], F32, tag=("pdo" if h % 2 == 0 else "pd")
                        )
                        nc.tensor.matmul(
                            pcd[:], vd_aug[:, h, :], probsTd[:, h, :],
                            start=True, stop=True,
                        )
                        dnd = rows.tile([1, B], F32, tag="dnd", bufs=2)
                        nc.vector.tensor_copy(out=dnd[:], in_=pcd[HD : HD + 1, :])
                        rcpd = rows.tile([1, B], F32, tag="rcpd", bufs=2)
                        nc.vector.reciprocal_approx_fast(out=rcpd[:], in_=dnd[:])
                        repd = rows.tile([HD, B], F32, tag="repd", bufs=2)
                        nc.gpsimd.partition_broadcast(repd[:], rcpd[:])
                        nc.vector.tensor_tensor(
                            out=ctxdT[off : off + 64, dc, :],
                            in0=pcd[0:HD, :], in1=repd[:], op=ALU.mult,
                        )
                        if h % 3 == 2:
                            yield

                    # dialog output projection + residual + LN
                    # (po shares the psO banks with the FFN po rotation;
                    #  the LN stat accumulators take the psD ping-pong banks,
                    #  which have no other users from here to the fixup.)
                    ydT = dlgp.tile([128, HC, B], F32R)
                    lnpd_m = psD.tile([1, B], F32, tag="pd")
                    lnpd_q = psD.tile([1, B], F32, tag="pdo")
                    for oc in range(HC):
                        po = psO.tile([128, B], F32, tag="po")
                        for hc in range(HC):
                            nc.tensor.matmul(
                                po[:], dwo_s[:, hc, oc * 128 : (oc + 1) * 128],
                                ctxdT[:, hc, :], start=(hc == 0), stop=(hc == HC - 1),
                            )
                        nc.vector.scalar_tensor_tensor(
                            out=ydT[:, oc, :], in0=po[:], scalar=dbo_s[:, oc : oc + 1],
                            in1=clsF[:, oc, :], op0=ALU.add, op1=ALU.add,
                        )
                        dsq = dlgp.tile([128, B], F32R, tag="dsq", bufs=2)
                        nc.vector.tensor_tensor(
                            out=dsq[:], in0=ydT[:, oc, :], in1=ydT[:, oc, :],
                            op=ALU.mult,
                        )
                        nc.tensor.matmul(
                            lnpd_m[:], ones_fr[:, 0:1], ydT[:, oc, :],
                            start=(oc == 0), stop=(oc == HC - 1),
                            skip_group_check=True,
                        )
                        nc.tensor.matmul(
                            lnpd_q[:], ones_fr[:, 0:1], dsq[:],
                            start=(oc == 0), stop=(oc == HC - 1),
                            skip_group_check=True,
                        )
                        yield
                    rstd_rep, msc_rep = _ln_reps(
                        nc, rows, lnpd_m, lnpd_q, eps_t, B, HID, oscale=WSC
                    )
                    for oc in range(HC):
                        nc.vector.tensor_tensor(
                            out=x2clsT[:, oc, :], in0=ydT[:, oc, :], in1=rstd_rep[:],
                            op=ALU.mult,
                        )
                        nc.vector.tensor_tensor(
                            out=x2clsT[:, oc, :], in0=x2clsT[:, oc, :], in1=msc_rep[:],
                            op=ALU.subtract,
                        )
                        if oc < 4:
                            nc.vector.tensor_copy(
                                out=x2q[:, oc, :], in_=x2clsT[:, oc, :]
                            )
                        if oc % 2 == 1:
                            yield

                x2clsT = dlgp.tile([128, HC, B], BF)
                x2q = dlgp.tile([128, 4, B], F8)

                _drive(ffn_seq(0, None))
                _drive(ffn_seq(1, None))
                _drive(ffn_seq(2, None), dialog_stage())
                # last seq's FFN with the CLS fixup riding along
                _drive(ffn_seq(SPC - 1, x2clsT))


def _build():
    nc = bacc.Bacc(
        "TRN2", target_bir_lowering=False, debug=False, num_devices=NCORES
    )
    d = {}
    d["xbf"] = nc.dram_tensor("xbf", [SPC, 128, HC, S], BF, kind="ExternalInput")[:]
    d["x8"] = nc.dram_tensor("x8", [SPC, 128, HC, S], F8, kind="ExternalInput")[:]
    for nm in ["dwq", "dwk", "dwv", "dwo"]:
        d[nm] = nc.dram_tensor(nm, [128, HC, HID], BF, kind="ExternalInput")[:]
    for nm in ["wq_hi", "wk_hi", "wv_hi", "wao_hi"]:
        d[nm] = nc.dram_tensor(nm, [128, HC, HID], F8, kind="ExternalInput")[:]
    for nm in ["bq", "bk", "bao", "dbq", "dbk", "dbo", "bo2"]:
        d[nm] = nc.dram_tensor(nm, [128, HC], F32, kind="ExternalInput")[:]
    d["bv_rep"] = nc.dram_tensor("bv_rep", [128, HID], BF, kind="ExternalInput")[:]
    d["dbv_rep"] = nc.dram_tensor("dbv_rep", [128, HID], BF, kind="ExternalInput")[:]
    d["bi"] = nc.dram_tensor("bi", [128, IC], F32, kind="ExternalInput")[:]
    d["wi8"] = nc.dram_tensor("wi8", [128, IC, 4, 128], F8, kind="ExternalInput")[:]
    d["wib"] = nc.dram_tensor("wib", [128, IC, 2, 128], BF, kind="ExternalInput")[:]
    d["wo2_hi"] = nc.dram_tensor(
        "wo2_hi", [128, HC, IC, 128], F8, kind="ExternalInput"
    )[:]
    d["cmaskt"] = nc.dram_tensor("cmaskt", [B, NH, B], F32, kind="ExternalInput")[:]
    d["out"] = nc.dram_tensor("out", [SPC, 128, HC, S], F32, kind="ExternalOutput")[:]

    with tile.TileContext(nc, num_cores=NCORES) as tc:
        _emit(tc, d)
    nc.compile()
    return nc


def _np_bf16():
    import ml_dtypes

    return ml_dtypes.bfloat16


def _np_fp8():
    import ml_dtypes

    return ml_dtypes.float8_e4m3


def _pack_w(w):
    BF_NP = _np_bf16()
    return np.ascontiguousarray(
        np.asarray(w, np.float32).reshape(HC, 128, HID).transpose(1, 0, 2)
    ).astype(BF_NP)


def _pack_b(b, nch=HC):
    return np.ascontiguousarray(np.asarray(b, np.float32).reshape(nch, 128).T)


def _make_cmaskt():
    # additive mask, transposed [key, query], replicated per head.
    # -30 stands in for the reference's -10000 (exp(-30) ~ 9e-14 is
    # negligible next to any unmasked term, and row 0 - where every
    # in-dialog entry is masked - still reduces to softmax(s) exactly);
    # cross-dialog pairs use -60 so they stay negligible even against
    # fully-masked rows.
    pos = np.arange(TURNS)
    base = (pos[None, :] >= pos[:, None]).astype(np.float32) * (-30.0)
    cm = np.full((B, B), -60.0, np.float32)
    for dd in range(NDLG):
        cm[dd * TURNS : (dd + 1) * TURNS, dd * TURNS : (dd + 1) * TURNS] = base
    cmt = cm.T  # [key, query]
    return np.ascontiguousarray(np.tile(cmt[:, None, :], (1, NH, 1)))


_NC = None


def _get_nc():
    global _NC
    if _NC is None:
        _NC = _build()
    return _NC


def _pack_w_hilo(w):
    """fp8 hi/lo split of WSC*w in the [128, HC, HID] stationary layout."""
    FP8 = _np_fp8()
    wf = np.ascontiguousarray(
        WSC * np.asarray(w, np.float32).reshape(HC, 128, HID).transpose(1, 0, 2)
    )
    hi = wf.astype(FP8)
    lo = (wf - hi.astype(np.float32)).astype(FP8)
    return hi, lo


def _prepare_in_maps(inputs):
    BF_NP = _np_bf16()
    FP8_NP = _np_fp8()
    f = lambda k: np.asarray(inputs[k], np.float32)
    shared = {
        "dwq": _pack_w(f("dWq")),
        "dwk": _pack_w(f("dWk")),
        "dwv": _pack_w(f("dWv")),
        "dwo": _pack_w(f("dWo")),
        "bq": _pack_b(WSC * f("bq")),
        "bk": _pack_b(WSC * f("bk")),
        "bao": _pack_b(WSC * WSC * f("bao")),
        "dbq": _pack_b(WSC * f("dbq")),
        "dbk": _pack_b(WSC * f("dbk")),
        "dbo": _pack_b(WSC * f("dbo")),
        "bo2": _pack_b(WSC * f("bo2")),
        "bv_rep": np.ascontiguousarray(
            np.tile(WSC * f("bv").reshape(1, HID), (128, 1))
        ).astype(BF_NP),
        "dbv_rep": np.ascontiguousarray(
            np.tile(WSC * f("dbv").reshape(1, HID), (128, 1))
        ).astype(BF_NP),
        "bi": _pack_b(f("bi"), IC),
        "cmaskt": _make_cmaskt(),
    }
    wif = WSC * f("Wi").reshape(HC, 128, IC, 128).transpose(1, 2, 0, 3)
    shared["wi8"] = np.ascontiguousarray(wif[:, :, 0:4]).astype(FP8_NP)
    shared["wib"] = np.ascontiguousarray(wif[:, :, 4:6]).astype(BF_NP)
    for nm, key in [("wq", "Wq"), ("wk", "Wk"), ("wv", "Wv"), ("wao", "Wao")]:
        shared[nm + "_hi"] = _pack_w_hilo(f(key))[0]
    wo2f = np.ascontiguousarray(
        WSC * f("Wo2").reshape(IC, 128, HC, 128).transpose(1, 2, 0, 3)
    )
    shared["wo2_hi"] = wo2f.astype(FP8_NP)
    x = np.asarray(inputs["hidden_states"], np.float32)
    in_maps = []
    for c in range(NCORES):
        xs = x[c * SPC : (c + 1) * SPC]  # [4, 512, 768]
        xp = np.ascontiguousarray(
            xs.transpose(0, 2, 1).reshape(SPC, HC, 128, S).transpose(0, 2, 1, 3)
        )
        in_maps.append(
            {
                **shared,
                # xbf carries WSC^2 (the AO residual add matches the
                # WSC^2-scaled attention-output psum); x8 is the unscaled
                # fp8 GEMM operand.
                "xbf": (WSC * WSC * xp).astype(BF_NP),
                "x8": xp.astype(FP8_NP),
            }
        )
    return in_maps


def _assemble(results):
    parts = []
    for c in range(NCORES):
        o = np.asarray(results[c]["out"], np.float32)  # [4, 128, 6, 512]
        parts.append(o.transpose(0, 2, 1, 3).reshape(SPC, HID, S).transpose(0, 2, 1))
    return np.ascontiguousarray(np.concatenate(parts, axis=0))


def run(inputs, trace=False):
    nc = _get_nc()
    in_maps = _prepare_in_maps(inputs)
    res = run_bass_kernel_spmd(
        nc, in_maps, core_ids=list(range(NCORES)), trace=trace
    )
    return _assemble(res.results), res


def kernel(**inputs):
    out, _ = run(inputs)
    return out
